# revision 12
# baseline (speedup 1.0000x reference)
"""Fake-quantized multi-head attention block on 8 TRN2 NeuronCores.

Data-parallel over batch (1 element per core); integer-domain quantized
matmuls in bf16; global fake-quant scales via tiny AllGather
collectives + local max.  Key structural points:

- A dummy AllGather fires at t=0 to absorb SPMD launch skew while
  stage-1 runs; later collectives then cost ~2-3us instead of ~12-27us.
- aug mega-tensors [128, 12*N]: rows 0:64 = q/k head slices (ints),
  aug_k rows 64:66 = ones, aug_q rows 64:66 = hi/lo split of
  ln(r*127/maxp)/catt written after AG#2, rows 66:128 zero.  Full
  128-row operands in BOTH attention phases keep the PE HAM clock-gate
  at full speed (64-row operands measurably halve the PE clock).
- v absmax rides its own early AllGather (AG#v) fired before phase A;
  v-quantize runs on gpsimd during phase A (gpsimd is otherwise idle).
- ln(r)/catt is computed and PE-transposed during AG#2 flight; the
  global -ln(maxp/127)/catt constant lands as one ACT bias add, and the
  hi/lo rows land in the aug tile with 2 strided DMAs.
"""

import sys

sys.path.insert(0, "/opt/trn_rl_repo")

import numpy as np
import ml_dtypes

import concourse.mybir as mybir
import concourse.tile as tile
import concourse.bass_isa as bass_isa
from concourse import bacc
from concourse.bass_utils import run_bass_kernel_spmd

f32 = mybir.dt.float32
bf16 = mybir.dt.bfloat16
i8 = mybir.dt.int8
ALU = mybir.AluOpType
ACT = mybir.ActivationFunctionType
AX = mybir.AxisListType
RED = bass_isa.ReduceOp

B, N, C = 8, 1024, 768
H, HD = 12, 64
NCORES = 8
MAGIC = float(np.float32(3 * 2**22))
CORES = list(range(NCORES))
RG = [CORES]

SFIN_CONST = [1.0]
CATT_CONST = [1.0]


def build_graph(inv_s_x: float):
    nc = bacc.Bacc("TRN2", target_bir_lowering=False, debug=False, num_devices=NCORES)

    xT_ext = nc.dram_tensor("xT", [C, N], i8, kind="ExternalInput")
    wq_qkv_ext = nc.dram_tensor("wq_qkv", [C, 3 * C], i8, kind="ExternalInput")
    wq_proj_ext = nc.dram_tensor("wq_proj", [C, C], i8, kind="ExternalInput")
    bqs_ext = nc.dram_tensor("bqs", [1, 3 * C], f32, kind="ExternalInput")
    bp_ext = nc.dram_tensor("bp", [1, C], f32, kind="ExternalInput")
    out_ext = nc.dram_tensor("out", [N, C], f32, kind="ExternalOutput")

    with tile.TileContext(nc) as tc:
        run_body(nc, tc, inv_s_x, xT_ext, wq_qkv_ext, wq_proj_ext, bqs_ext, bp_ext, out_ext)
    nc.finalize()
    return nc


def run_body(nc, tc, inv_s_x, xT_ext, wq_qkv_ext, wq_proj_ext, bqs_ext, bp_ext, out_ext):
    with (
        tc.tile_pool(name="persist", bufs=1) as pp,
        tc.tile_pool(name="dram", bufs=1, space="DRAM") as dram,
    ):
        # aug mega-tensors: column block h*N..(h+1)*N is head h
        aug_q = pp.tile([128, H * N], bf16, tag="aug_q", name="aug_q")
        aug_k = pp.tile([128, H * N], bf16, tag="aug_k", name="aug_k")
        zbuf = pp.tile([128, 96], f32, tag="zbuf", name="zbuf")
        mlbuf = pp.tile([128, 96], f32, tag="mlbuf", name="mlbuf")
        qkv_abs = pp.tile([128, 24], f32, tag="qkv_abs", name="qkv_abs")
        v_abs = pp.tile([128, 16], f32, tag="v_abs", name="v_abs")
        o_abs = pp.tile([128, 6], f32, tag="o_abs", name="o_abs")
        sc = pp.tile([128, 16], f32, tag="sc", name="sc")
        bqs_cols = pp.tile([128, 12], f32, tag="bqs_cols", name="bqs_cols")
        bv_bc = pp.tile([128, C], f32, tag="bv_bc", name="bv_bc")
        hi_rows = pp.tile([H, N], bf16, tag="hi_rows", name="hi_rows")
        lo_rows = pp.tile([H, N], bf16, tag="lo_rows", name="lo_rows")
        magic_col = pp.tile([128, 1], f32, tag="magic_col", name="magic_col")
        nmagic_col = pp.tile([128, 1], f32, tag="nmagic_col", name="nmagic_col")

        ar0_in = dram.tile([1, 1], f32, tag="ar0_in", name="ar0_in")
        ar0_out = dram.tile([1, 8], f32, tag="ar0_out", name="ar0_out")
        arm2_in = dram.tile([1, 1], f32, tag="arm2_in", name="arm2_in")
        arm2_out = dram.tile([1, 8], f32, tag="arm2_out", name="arm2_out")
        ar1_in = dram.tile([1, 2], f32, tag="ar1_in", name="ar1_in")
        ar1_out = dram.tile([1, 2], f32, tag="ar1_out", name="ar1_out")
        arv_in = dram.tile([1, 1], f32, tag="arv_in", name="arv_in")
        arv_out = dram.tile([1, 1], f32, tag="arv_out", name="arv_out")
        ar2_in = dram.tile([1, 1], f32, tag="ar2_in", name="ar2_in")
        ar2_out = dram.tile([1, 1], f32, tag="ar2_out", name="ar2_out")
        ar3_in = dram.tile([1, 1], f32, tag="ar3_in", name="ar3_in")
        ar3_out = dram.tile([1, 1], f32, tag="ar3_out", name="ar3_out")

        nc.vector.memset(magic_col[:], MAGIC)

        # per-output-channel qkv bias columns: one strided DMA, issued
        # before anything else queues on gpsimd
        nc.gpsimd.dma_start(
            bqs_cols[:, 0:12],
            bqs_ext[0:1, 0:1536].rearrange("a (t p) -> (a p) t", p=128),
        )

        # PE warm-up burst (ramp the PE clock) while DMAs fly
        wut = pp.tile([128, 512], bf16, tag="wut", name="wut")
        nc.vector.memset(wut[:], 1.0)
        with tc.tile_pool(name="pswu", bufs=1, space="PSUM") as pswu:
            wps = pswu.tile([128, 512], f32, tag="wps", name="wps")
            for _ in range(24):
                nc.tensor.matmul(wps[:], lhsT=wut[:, 0:128], rhs=wut[:], start=True, stop=True)
        nc.vector.memset(nmagic_col[:], -MAGIC)
        with tc.tile_pool(name="brow", bufs=1) as br:
            bvrow = br.tile([1, C], f32, tag="bvrow", name="bvrow")
            nc.sync.dma_start(bvrow[:], bqs_ext[0:1, 1536:2304])
            nc.gpsimd.partition_broadcast(bv_bc[:], bvrow[:])

        # late pool: tensors born mid-kernel (vq during phase A, bp_bc at proj)
        with tc.tile_pool(name="late", bufs=1) as lp:
         vq = [lp.tile([128, C], bf16, tag=f"vq{t}", name=f"vq{t}") for t in range(8)]
         bp_bc = lp.tile([128, C], f32, tag="bp_bc", name="bp_bc")
         # v_f persists until v-quant (on gpsimd during phase A)
         with tc.tile_pool(name="vf_pool", bufs=1) as vfp:
          v_f = [vfp.tile([128, C], f32, tag=f"vf{t}", name=f"vf{t}") for t in range(8)]

          # ---- stage 1+2: x quant, QKV matmuls, AG#1 (q,k), quantize -----
          with (
            tc.tile_pool(name="wload", bufs=1) as wl,
            tc.tile_pool(name="qkvf_pool", bufs=1) as qp,
            tc.tile_pool(name="s12", bufs=2) as s12,
            tc.tile_pool(name="psq", bufs=4, space="PSUM") as psq,
          ):
            wq_bf = [wl.tile([128, 3 * C], bf16, tag=f"wq{t}", name=f"wq{t}") for t in range(6)]
            xq = [wl.tile([128, N], bf16, tag=f"xq{t}", name=f"xq{t}") for t in range(6)]
            qkv_f = [qp.tile([128, N], f32, tag=f"qkvf{t}", name=f"qkvf{t}") for t in range(12)]

            # int8 inputs: DMA then upcast to bf16 (x-quant happened on host)
            for t in range(6):
                x8 = wl.tile([128, N], i8, tag="x8", name=f"x8{t}", bufs=2)
                wq8 = wl.tile([128, 3 * C], i8, tag="wq8", name=f"wq8{t}", bufs=2)
                nc.sync.dma_start(x8[:], xT_ext[t * 128 : (t + 1) * 128, :])
                nc.sync.dma_start(wq8[:], wq_qkv_ext[t * 128 : (t + 1) * 128, :])
                nc.gpsimd.tensor_copy(xq[t][:], x8[:])
                nc.vector.tensor_copy(wq_bf[t][:], wq8[:])

            # q/k part: per-mt accumulation over kt
            for mt in range(12):
                pss = [psq.tile([128, 512], f32, tag="psq", name="psq", bufs=6) for _ in range(2)]
                for kt in range(6):
                    for nk in range(2):
                        mi = nc.tensor.matmul(
                            pss[nk][:],
                            lhsT=wq_bf[kt][:, mt * 128 : (mt + 1) * 128],
                            rhs=xq[kt][:, nk * 512 : (nk + 1) * 512],
                            start=(kt == 0),
                            stop=(kt == 5),
                            skip_group_check=True,
                        )
                        if nk == 1:
                            mi.ins.ldweights = False
                for nk in range(2):
                    nc.scalar.activation(
                        qkv_f[mt][:, nk * 512 : (nk + 1) * 512],
                        pss[nk][:],
                        ACT.Identity,
                        bias=bqs_cols[:, mt : mt + 1],
                    )
                    nc.vector.tensor_reduce(
                        qkv_abs[:, mt * 2 + nk : mt * 2 + nk + 1],
                        qkv_f[mt][:, nk * 512 : (nk + 1) * 512],
                        axis=AX.X,
                        op=ALU.max,
                        apply_absolute_value=True,
                    )

            # ---- AG#1: global absmax of q, k --------------------------------
            am2 = s12.tile([128, 2], f32, tag="am2", name="am2")
            nc.vector.tensor_reduce(am2[:, 0:1], qkv_abs[:, 0:12], axis=AX.X, op=ALU.max)
            nc.vector.tensor_reduce(am2[:, 1:2], qkv_abs[:, 12:24], axis=AX.X, op=ALU.max)
            am2r = s12.tile([128, 2], f32, tag="am2r", name="am2r")
            nc.gpsimd.partition_all_reduce(am2r[:], am2[:], 128, RED.max)
            nc.gpsimd.dma_start(ar1_in[:], am2r[0:1, :])
            nc.gpsimd.collective_compute(
                "AllReduce", ALU.max, replica_groups=RG, ins=[ar1_in.opt()], outs=[ar1_out.opt()]
            )
            g2 = pp.tile([1, 2], f32, tag="g2", name="g2")
            nc.gpsimd.dma_start(g2[:], ar1_out[0:1, 0:2])
            g2b = pp.tile([128, 2], f32, tag="g2b", name="g2b")
            nc.gpsimd.partition_broadcast(g2b[:], g2[:])

            # aug zero/one fills: per-head chunks, emitted after the AG#1
            # fire so they don't clog the gpsimd queue ahead of it
            for h in range(H):
                nc.gpsimd.memset(aug_q[64:128, h * N : (h + 1) * N], 0.0)
                nc.gpsimd.memset(aug_k[64:128, h * N : (h + 1) * N], 0.0)
                nc.gpsimd.memset(aug_k[64:66, h * N : (h + 1) * N], 1.0)

            inv2 = pp.tile([128, 2], f32, tag="inv2", name="inv2")
            nc.vector.reciprocal(inv2[:], g2b[:])
            nc.vector.tensor_scalar(inv2[:], inv2[:], 127.0, None, ALU.mult)
            nc.vector.tensor_tensor(sc[:, 3:4], g2b[:, 0:1], g2b[:, 1:2], ALU.mult)
            nc.vector.tensor_scalar(sc[:, 3:4], sc[:, 3:4], CATT_CONST[0], None, ALU.mult)
            nc.vector.reciprocal(sc[:, 9:10], sc[:, 3:4])

            # v matmuls fill the PE idle window during/after AG#1 (their
            # absmax rides the separate AG#v)
            for nt in range(8):
                pss = []
                for ick, (ck, cw) in enumerate(((0, 512), (512, 256))):
                    pss.append((psq.tile([128, 512], f32, tag="psq", name="psv", bufs=6), ck, cw))
                for kt in range(6):
                    for ick2, (ps, ck, cw) in enumerate(pss):
                        mi = nc.tensor.matmul(
                            ps[:, 0:cw],
                            lhsT=xq[kt][:, nt * 128 : (nt + 1) * 128],
                            rhs=wq_bf[kt][:, 1536 + ck : 1536 + ck + cw],
                            start=(kt == 0),
                            stop=(kt == 5),
                            skip_group_check=True,
                        )
                        if ick2 == 1:
                            mi.ins.ldweights = False
                for ick, (ps, ck, cw) in enumerate(pss):
                    nc.vector.scalar_tensor_tensor(
                        v_f[nt][:, ck : ck + cw],
                        ps[:, 0:cw],
                        1.0,
                        bv_bc[:, ck : ck + cw],
                        ALU.mult,
                        ALU.add,
                    )
                    nc.vector.tensor_reduce(
                        v_abs[:, nt * 2 + ick : nt * 2 + ick + 1],
                        v_f[nt][:, ck : ck + cw],
                        axis=AX.X,
                        op=ALU.max,
                        apply_absolute_value=True,
                    )

            # ---- AG#v: global absmax of v (fire before phase A) -------------
            vam = s12.tile([128, 1], f32, tag="vam", name="vam")
            nc.vector.tensor_reduce(vam[:], v_abs[:], axis=AX.X, op=ALU.max)
            vamr = s12.tile([128, 1], f32, tag="vamr", name="vamr")
            nc.gpsimd.partition_all_reduce(vamr[:], vam[:], 128, RED.max)
            nc.gpsimd.dma_start(arv_in[:], vamr[0:1, :])
            nc.gpsimd.collective_compute(
                "AllReduce", ALU.max, replica_groups=RG, ins=[arv_in.opt()], outs=[arv_out.opt()]
            )
            gv = pp.tile([1, 1], f32, tag="gv", name="gv")
            nc.gpsimd.dma_start(gv[:], arv_out[0:1, 0:1])
            nc.gpsimd.partition_broadcast(sc[:, 14:15], gv[:])
            nc.vector.reciprocal(sc[:, 15:16], sc[:, 14:15])
            nc.vector.tensor_scalar(sc[:, 15:16], sc[:, 15:16], 127.0, None, ALU.mult)

            # ---- quantize q/k into the aug mega-tiles ----------------------
            for i, mt in enumerate((0, 6, 1, 7, 2, 8, 3, 9, 4, 10, 5, 11)):
                inv = inv2[:, 0:1] if mt < 6 else inv2[:, 1:2]
                y = s12.tile([128, N], f32, tag="s12y", name="yq", bufs=2)
                nc.gpsimd.tensor_scalar(y[:], qkv_f[mt][:], inv, MAGIC, ALU.mult, ALU.add)
                qsc = s12.tile([128, N], bf16, tag="qsc", name="qsc", bufs=2)
                nc.gpsimd.tensor_scalar(qsc[:], y[:], MAGIC, None, ALU.subtract)
                dst = aug_q if mt < 6 else aug_k
                tt = mt if mt < 6 else mt - 6
                nc.sync.dma_start(dst[0:64, (2 * tt) * N : (2 * tt + 1) * N], qsc[0:64, :])
                nc.sync.dma_start(dst[0:64, (2 * tt + 1) * N : (2 * tt + 2) * N], qsc[64:128, :])


          # ---- phase A: attn[i,j] logits stats -----------------------------
          with (
            tc.tile_pool(name="phA", bufs=4) as pa,
            tc.tile_pool(name="psA", bufs=3, space="PSUM") as psa,
          ):
            armid_dummy = pp.tile([1, 8], f32, tag="armid_dummy", name="armid_dummy")
            for h in range(H):
                if h == 6:
                    # resync collective: input depends on head-5 stats so the
                    # fire can't be hoisted before mid-phase-A; absorbs
                    # inter-core drift under the compute shadow
                    nc.gpsimd.dma_start(ar0_in[:], zbuf[0:1, 47:48])
                    nc.gpsimd.collective_compute(
                        "AllGather", ALU.bypass, replica_groups=RG,
                        ins=[ar0_in.opt()], outs=[ar0_out.opt()],
                    )
                if h == 9:
                    nc.gpsimd.dma_start(armid_dummy[:], ar0_out[0:1, :])
                for it in range(8):
                    psl = psa.tile([128, N], f32, tag="psl", name="psl")
                    for jc in range(2):
                        mi = nc.tensor.matmul(
                            psl[:, jc * 512 : (jc + 1) * 512],
                            lhsT=aug_q[:, h * N + it * 128 : h * N + (it + 1) * 128],
                            rhs=aug_k[:, h * N + jc * 512 : h * N + (jc + 1) * 512],
                            start=True,
                            stop=True,
                        )
                        if jc == 1:
                            mi.ins.ldweights = False
                    col = h * 8 + it
                    ea = pa.tile([128, N], bf16, tag="ea", name="ea")
                    nc.scalar.activation(
                        ea[:], psl[:], ACT.Exp, scale=sc[:, 3:4],
                        accum_out=zbuf[:, col : col + 1],
                    )
                    nc.vector.tensor_reduce(mlbuf[:, col : col + 1], psl[:], axis=AX.X, op=ALU.max)
                    wps2 = psa.tile([128, 512], f32, tag="wps2", name="wps2", bufs=2)
                    nc.tensor.matmul(wps2[:], lhsT=wut[:, 0:128], rhs=ea[:, 0:512], start=True, stop=True)

          # ---- AG#2: max prob; ln(r)/c rows --------------------------------
          with (
            tc.tile_pool(name="phR", bufs=1) as pr,
            tc.tile_pool(name="psT", bufs=1, space="PSUM") as pst,
          ):
            from concourse.masks import make_identity

            maxe = pr.tile([128, 96], f32, tag="maxe", name="maxe")
            nc.scalar.activation(maxe[:], mlbuf[:], ACT.Exp, scale=sc[:, 3:4])
            rz = pr.tile([128, 96], f32, tag="rz", name="rz")
            nc.vector.reciprocal(rz[:], zbuf[:])
            mp = pr.tile([128, 96], f32, tag="mp", name="mp")
            nc.vector.tensor_tensor(mp[:], maxe[:], rz[:], ALU.mult)
            pk1 = pr.tile([128, 1], f32, tag="pk1", name="pk1")
            nc.vector.tensor_reduce(pk1[:], mp[:], axis=AX.X, op=ALU.max)
            pk1r = pr.tile([128, 1], f32, tag="pk1r", name="pk1r")
            nc.gpsimd.partition_all_reduce(pk1r[:], pk1[:], 128, RED.max)
            nc.gpsimd.dma_start(ar2_in[:], pk1r[0:1, :])
            nc.gpsimd.collective_compute(
                "AllReduce", ALU.max, replica_groups=RG, ins=[ar2_in.opt()], outs=[ar2_out.opt()]
            )

            # -- PE keep-warm during AG#2 window ----------------------------
            wpsw = pst.tile([128, 512], f32, tag="wpsw", name="wpsw", bufs=2)
            wpsw2 = pst.tile([128, 512], f32, tag="wpsw", name="wpsw2", bufs=2)
            for wi in range(8):
                nc.tensor.matmul(
                    (wpsw if wi % 2 == 0 else wpsw2)[:],
                    lhsT=wut[:, 0:128], rhs=wut[:], start=True, stop=True,
                )

            # -- during AG#2 flight: v-quant (needs only AG#v result) --------
            for nt in range(8):
                yv = pr.tile([128, C], f32, tag="yv", name="yv", bufs=2)
                nc.scalar.activation(yv[:], v_f[nt][:], ACT.Identity, bias=magic_col[:], scale=sc[:, 15:16])
                nc.vector.tensor_scalar(vq[nt][:], yv[:], MAGIC, None, ALU.subtract)

            # -- during AG#2 flight: ln(r)/catt, transposed ------------------
            lnr = pr.tile([128, 96], f32, tag="lnr", name="lnr")
            nc.scalar.activation(lnr[:], rz[:], ACT.Ln)
            lnrc = pr.tile([128, 128], f32, tag="lnrc", name="lnrc")
            nc.vector.memset(lnrc[:], 0.0)
            nc.vector.tensor_scalar(lnrc[:, 0:96], lnr[:], sc[:, 9:10], None, ALU.mult)
            idn = pr.tile([128, 128], f32, tag="idn", name="idn")
            make_identity(nc, idn[:])
            psT = pst.tile([128, 128], f32, tag="psT", name="psT")
            nc.tensor.transpose(psT[:], lnrc[:], idn[:])
            lnrcT_pre = pr.tile([128, 128], f32, tag="lnrcT_pre", name="lnrcT_pre")
            nc.scalar.activation(lnrcT_pre[:], psT[:], ACT.Copy)

            # -- AG#2 result: maxp_g ----------------------------------------
            g2p = pr.tile([1, 1], f32, tag="g2p", name="g2p")
            nc.gpsimd.dma_start(g2p[:], ar2_out[0:1, 0:1])
            nc.gpsimd.partition_broadcast(sc[:, 7:8], g2p[:])
            nc.vector.reciprocal(sc[:, 8:9], sc[:, 7:8])
            nc.vector.tensor_scalar(sc[:, 8:9], sc[:, 8:9], 127.0, None, ALU.mult)
            # one strided DMA: [96,128] -> [12, 1024]; runs during AG#2 flight
            lnrc_rows = pr.tile([H, N], f32, tag="lnrc_rows", name="lnrc_rows")
            nc.scalar.dma_start(lnrc_rows[:], lnrcT_pre[0:96, 0:128])
            # cterm = ln(127/maxp)/catt folded into the hi/lo split directly
            cterm = pr.tile([128, 1], f32, tag="cterm", name="cterm")
            nc.scalar.activation(cterm[:], sc[:, 8:9], ACT.Ln)
            nc.vector.tensor_tensor(cterm[:], cterm[:], sc[:, 9:10], ALU.mult)
            nc.vector.tensor_scalar(hi_rows[:], lnrc_rows[:], cterm[0:H, 0:1], None, ALU.add)
            nc.vector.scalar_tensor_tensor(
                lo_rows[:], lnrc_rows[:], cterm[0:H, 0:1], hi_rows[:], ALU.add, ALU.subtract
            )
            # two strided DMAs land hi/lo into aug_q rows 64/65
            nc.scalar.dma_start(aug_q[64:65, :], hi_rows[:])
            nc.scalar.dma_start(aug_q[65:66, :], lo_rows[:])

         # ---- phase B: quantized probs + PV (zero-padded vz operands) ----
         with tc.tile_pool(name="oint_pool", bufs=1) as op_:
           o_int = [op_.tile([128, N], f32, tag=f"oint{t}", name=f"oint{t}") for t in range(6)]
           wp_bf = [op_.tile([128, C], bf16, tag=f"wp{t}", name=f"wp{t}") for t in range(6)]
           wp8 = [op_.tile([128, C], i8, tag=f"wp8{t}", name=f"wp8{t}") for t in range(6)]
           # prefetch proj weights + bias during phase B (gpsimd queue)
           for t in range(6):
               nc.gpsimd.dma_start(wp8[t][:], wq_proj_ext[t * 128 : (t + 1) * 128, :])
               nc.gpsimd.tensor_copy(wp_bf[t][:], wp8[t][:])
           with tc.tile_pool(name="brow2", bufs=1) as br2:
               bprow = br2.tile([1, C], f32, tag="bprow", name="bprow")
               nc.gpsimd.dma_start(bprow[:], bp_ext[:])
               nc.gpsimd.partition_broadcast(bp_bc[:], bprow[:])
           with (
             tc.tile_pool(name="phB", bufs=4) as pb,
             tc.tile_pool(name="vzp", bufs=2) as vzp,
             tc.tile_pool(name="psB", bufs=2, space="PSUM") as psb,
             tc.tile_pool(name="psO", bufs=2, space="PSUM") as pso_pool,
           ):
             armid2_dummy = pp.tile([1, 8], f32, tag="armid2_dummy", name="armid2_dummy")
             for hp in range(6):
                 if hp == 3:
                     nc.gpsimd.dma_start(arm2_in[:], o_abs[0:1, 2:3])
                     nc.gpsimd.collective_compute(
                         "AllGather", ALU.bypass, replica_groups=RG,
                         ins=[arm2_in.opt()], outs=[arm2_out.opt()],
                     )
                 if hp == 5:
                     nc.gpsimd.dma_start(armid2_dummy[:], arm2_out[0:1, :])
                 h0, h1 = 2 * hp, 2 * hp + 1
                 # padded PV weights: vz[:, jt*256 + 0:64] = v cols of h0,
                 # vz[:, jt*256 + 192:256] = v cols of h1, rest zero.
                 vz = vzp.tile([128, 8 * 256], bf16, tag="vz", name="vz")
                 nc.gpsimd.memset(vz[:], 0.0)
                 for jt in range(8):
                     nc.sync.dma_start(
                         vz[:, jt * 256 : jt * 256 + 64], vq[jt][:, h0 * 64 : (h0 + 1) * 64]
                     )
                     nc.sync.dma_start(
                         vz[:, jt * 256 + 192 : jt * 256 + 256], vq[jt][:, h1 * 64 : (h1 + 1) * 64]
                     )
                 pso = pso_pool.tile([128, N], f32, tag="pso", name="pso")
                 for jt in range(8):
                     pqs = []
                     for h in (h0, h1):
                         pslT = psb.tile([128, N], f32, tag="pslT", name="pslT")
                         for ic in range(2):
                             mi = nc.tensor.matmul(
                                 pslT[:, ic * 512 : (ic + 1) * 512],
                                 lhsT=aug_k[:, h * N + jt * 128 : h * N + (jt + 1) * 128],
                                 rhs=aug_q[:, h * N + ic * 512 : h * N + (ic + 1) * 512],
                                 start=True,
                                 stop=True,
                             )
                             if ic == 1:
                                 mi.ins.ldweights = False
                         ep = pb.tile([128, N], f32, tag="ep", name="ep")
                         nc.scalar.activation(ep[:], pslT[:], ACT.Exp, scale=sc[:, 3:4])
                         pq = pb.tile([128, N], bf16, tag="pq", name="pq")
                         nc.vector.tensor_scalar(pq[:], ep[:], MAGIC, MAGIC, ALU.add, ALU.subtract)
                         pqs.append(pq)
                     for hh, pq in enumerate(pqs):
                         for ic in range(2):
                             mi = nc.tensor.matmul(
                                 pso[:, ic * 512 : (ic + 1) * 512],
                                 lhsT=vz[:, jt * 256 + hh * 128 : jt * 256 + (hh + 1) * 128],
                                 rhs=pq[:, ic * 512 : (ic + 1) * 512],
                                 start=(jt == 0 and hh == 0),
                                 stop=(jt == 7 and hh == 1),
                                 skip_group_check=True,
                             )
                             if ic == 1:
                                 mi.ins.ldweights = False
                 nc.vector.tensor_copy(o_int[hp][:], pso[:])
                 nc.vector.tensor_reduce(
                     o_abs[:, hp : hp + 1], o_int[hp][:], axis=AX.X, op=ALU.max, apply_absolute_value=True
                 )

           # ---- AG#3 + quantize o + proj ----------------------------------
           with (
             tc.tile_pool(name="phC", bufs=3) as pc,
             tc.tile_pool(name="oq_pool", bufs=1) as oqp,
             tc.tile_pool(name="psF", bufs=4, space="PSUM") as psf_pool,
           ):
             oam = pc.tile([128, 1], f32, tag="oam", name="oam")
             nc.vector.tensor_reduce(oam[:], o_abs[:], axis=AX.X, op=ALU.max)
             oamr = pc.tile([128, 1], f32, tag="oamr", name="oamr")
             nc.gpsimd.partition_all_reduce(oamr[:], oam[:], 128, RED.max)
             nc.gpsimd.dma_start(ar3_in[:], oamr[0:1, :])
             nc.gpsimd.collective_compute(
                 "AllReduce", ALU.max, replica_groups=RG, ins=[ar3_in.opt()], outs=[ar3_out.opt()]
             )
             g3 = pc.tile([1, 1], f32, tag="g3", name="g3")
             nc.gpsimd.dma_start(g3[:], ar3_out[0:1, 0:1])
             nc.gpsimd.partition_broadcast(sc[:, 10:11], g3[:])

             nc.vector.reciprocal(sc[:, 11:12], sc[:, 10:11])
             nc.vector.tensor_scalar(sc[:, 11:12], sc[:, 11:12], 127.0, None, ALU.mult)
             nc.vector.tensor_tensor(sc[:, 12:13], sc[:, 7:8], sc[:, 14:15], ALU.mult)
             nc.vector.tensor_tensor(sc[:, 12:13], sc[:, 12:13], sc[:, 10:11], ALU.mult)
             nc.vector.tensor_scalar(sc[:, 12:13], sc[:, 12:13], SFIN_CONST[0], None, ALU.mult)

             oq = [oqp.tile([128, N], bf16, tag=f"oq{t}", name=f"oq{t}") for t in range(6)]
             for t in range(6):
                 if t % 2 == 0:
                     y = pc.tile([128, N], f32, tag="yo", name="yo")
                     nc.scalar.activation(y[:], o_int[t][:], ACT.Identity, bias=magic_col[:], scale=sc[:, 11:12])
                     nc.vector.tensor_scalar(oq[t][:], y[:], MAGIC, None, ALU.subtract)
                 else:
                     y = pc.tile([128, N], f32, tag="yo", name="yo")
                     nc.vector.tensor_scalar(y[:], o_int[t][:], sc[:, 11:12], MAGIC, ALU.mult, ALU.add)
                     nc.vector.tensor_scalar(oq[t][:], y[:], MAGIC, None, ALU.subtract)

             for g in range(2):
                 psfs = [psf_pool.tile([128, C], f32, tag="psf", name="psf") for _ in range(4)]
                 if g == 0:
                     for wi in range(10):
                         nc.tensor.matmul(
                             psfs[wi % 2][:, 0:512],
                             lhsT=wut[:, 0:128], rhs=wut[:], start=True, stop=True,
                         )
                 for kt in range(6):
                     for nn in range(4):
                         nt = g * 4 + nn
                         for ick2, (ck, cw) in enumerate(((0, 512), (512, 256))):
                             mi = nc.tensor.matmul(
                                 psfs[nn][:, ck : ck + cw],
                                 lhsT=oq[kt][:, nt * 128 : (nt + 1) * 128],
                                 rhs=wp_bf[kt][:, ck : ck + cw],
                                 start=(kt == 0),
                                 stop=(kt == 5),
                                 skip_group_check=True,
                             )
                             if ick2 == 1:
                                 mi.ins.ldweights = False
                 for nn in range(4):
                     nt = g * 4 + nn
                     ot = pc.tile([128, C], f32, tag="ot", name="ot")
                     nc.vector.scalar_tensor_tensor(
                         ot[:], psfs[nn][:], sc[:, 12:13], bp_bc[:], ALU.mult, ALU.add
                     )
                     nc.sync.dma_start(out_ext[nt * 128 : (nt + 1) * 128, :], ot[:])


def _host_prep(x, w_qkv, b_qkv, w_proj, b_proj):
    x = np.asarray(x, dtype=np.float32)
    w_qkv = np.asarray(w_qkv, dtype=np.float32)
    b_qkv = np.asarray(b_qkv, dtype=np.float32)
    w_proj = np.asarray(w_proj, dtype=np.float32)
    b_proj = np.asarray(b_proj, dtype=np.float32)

    qmax = np.float32(127.0)
    s_x = np.maximum(np.max(np.abs(x)) / qmax, np.float32(1e-8))
    s_wq = np.maximum(np.max(np.abs(w_qkv)) / qmax, np.float32(1e-8))
    s_wp = np.maximum(np.max(np.abs(w_proj)) / qmax, np.float32(1e-8))
    inv_s_x = float(np.float32(1.0) / s_x)

    wq_qkv = np.round(w_qkv / s_wq).astype(np.int8)
    wq_proj = np.round(w_proj / s_wp).astype(np.int8)
    bqs = (b_qkv / (s_x * s_wq)).astype(np.float32)[None, :]
    bp = b_proj.astype(np.float32)[None, :]

    sxw = float(s_x) * float(s_wq)
    sfin = float(s_wp) * sxw / (127.0**3)
    catt = 0.125 * sxw * sxw / (127.0 * 127.0)
    inv32 = np.float32(inv_s_x)
    in_maps = [
        {
            "xT": np.round(np.ascontiguousarray(x[b].T) * inv32).astype(np.int8),
            "wq_qkv": wq_qkv,
            "wq_proj": wq_proj,
            "bqs": bqs,
            "bp": bp,
        }
        for b in range(B)
    ]
    return inv_s_x, sfin, catt, in_maps


_CACHE = {}


def kernel(x, w_qkv, b_qkv, w_proj, b_proj):
    inv_s_x, sfin, catt, in_maps = _host_prep(x, w_qkv, b_qkv, w_proj, b_proj)
    key = (inv_s_x, sfin, catt)
    if key not in _CACHE:
        SFIN_CONST[0] = sfin
        CATT_CONST[0] = catt
        _CACHE[key] = build_graph(inv_s_x)
    nc = _CACHE[key]
    res = run_bass_kernel_spmd(nc, in_maps, CORES)
    return np.stack([res.results[b]["out"] for b in range(B)], axis=0)


def build_and_inmaps(x, w_qkv, b_qkv, w_proj, b_proj):
    inv_s_x, sfin, catt, in_maps = _host_prep(x, w_qkv, b_qkv, w_proj, b_proj)
    SFIN_CONST[0] = sfin
    CATT_CONST[0] = catt
    nc = build_graph(inv_s_x)
    return nc, in_maps


# revision 13
# speedup vs baseline: 1.3773x; 1.3773x over previous
"""Fake-quantized multi-head attention block on 8 TRN2 NeuronCores.

Data-parallel over batch (1 element per core); integer-domain quantized
matmuls in bf16; global fake-quant scales via tiny AllGather
collectives + local max.  Key structural points:

- A dummy AllGather fires at t=0 to absorb SPMD launch skew while
  stage-1 runs; later collectives then cost ~2-3us instead of ~12-27us.
- aug mega-tensors [128, 12*N]: rows 0:64 = q/k head slices (ints),
  aug_k rows 64:66 = ones, aug_q rows 64:66 = hi/lo split of
  ln(r*127/maxp)/catt written after AG#2, rows 66:128 zero.  Full
  128-row operands in BOTH attention phases keep the PE HAM clock-gate
  at full speed (64-row operands measurably halve the PE clock).
- v absmax rides its own early AllGather (AG#v) fired before phase A;
  v-quantize runs on gpsimd during phase A (gpsimd is otherwise idle).
- ln(r)/catt is computed and PE-transposed during AG#2 flight; the
  global -ln(maxp/127)/catt constant lands as one ACT bias add, and the
  hi/lo rows land in the aug tile with 2 strided DMAs.
"""

import sys

sys.path.insert(0, "/opt/trn_rl_repo")

import numpy as np
import ml_dtypes

import concourse.mybir as mybir
import concourse.tile as tile
import concourse.bass_isa as bass_isa
from concourse import bacc
from concourse.bass_utils import run_bass_kernel_spmd

f32 = mybir.dt.float32
bf16 = mybir.dt.bfloat16
i8 = mybir.dt.int8
ALU = mybir.AluOpType
ACT = mybir.ActivationFunctionType
AX = mybir.AxisListType
RED = bass_isa.ReduceOp

B, N, C = 8, 1024, 768
H, HD = 12, 64
NCORES = 8
MAGIC = float(np.float32(3 * 2**22))
CORES = list(range(NCORES))
RG = [CORES]

SFIN_CONST = [1.0]
CATT_CONST = [1.0]


def build_graph(inv_s_x: float):
    nc = bacc.Bacc("TRN2", target_bir_lowering=False, debug=False, num_devices=NCORES)

    xT_ext = nc.dram_tensor("xT", [C, N], i8, kind="ExternalInput")
    wq_qkv_ext = nc.dram_tensor("wq_qkv", [C, 3 * C], i8, kind="ExternalInput")
    wq_proj_ext = nc.dram_tensor("wq_proj", [C, C], i8, kind="ExternalInput")
    bqs_ext = nc.dram_tensor("bqs", [1, 3 * C], f32, kind="ExternalInput")
    bp_ext = nc.dram_tensor("bp", [1, C], f32, kind="ExternalInput")
    out_ext = nc.dram_tensor("out", [N, C], f32, kind="ExternalOutput")

    with tile.TileContext(nc) as tc:
        run_body(nc, tc, inv_s_x, xT_ext, wq_qkv_ext, wq_proj_ext, bqs_ext, bp_ext, out_ext)
    nc.finalize()
    return nc


def run_body(nc, tc, inv_s_x, xT_ext, wq_qkv_ext, wq_proj_ext, bqs_ext, bp_ext, out_ext):
    with (
        tc.tile_pool(name="persist", bufs=1) as pp,
        tc.tile_pool(name="dram", bufs=1, space="DRAM") as dram,
    ):
        # aug mega-tensors: column block h*N..(h+1)*N is head h
        aug_q = pp.tile([128, H * N], bf16, tag="aug_q", name="aug_q")
        aug_k = pp.tile([128, H * N], bf16, tag="aug_k", name="aug_k")
        zbuf = pp.tile([128, 96], f32, tag="zbuf", name="zbuf")
        mlbuf = pp.tile([128, 96], f32, tag="mlbuf", name="mlbuf")
        qkv_abs = pp.tile([128, 24], f32, tag="qkv_abs", name="qkv_abs")
        v_abs = pp.tile([128, 16], f32, tag="v_abs", name="v_abs")
        o_abs = pp.tile([128, 6], f32, tag="o_abs", name="o_abs")
        sc = pp.tile([128, 16], f32, tag="sc", name="sc")
        bqs_cols = pp.tile([128, 12], f32, tag="bqs_cols", name="bqs_cols")
        bv_bc = pp.tile([128, C], f32, tag="bv_bc", name="bv_bc")
        hi_rows = pp.tile([H, N], bf16, tag="hi_rows", name="hi_rows")
        lo_rows = pp.tile([H, N], bf16, tag="lo_rows", name="lo_rows")
        magic_col = pp.tile([128, 1], f32, tag="magic_col", name="magic_col")
        nmagic_col = pp.tile([128, 1], f32, tag="nmagic_col", name="nmagic_col")

        ar0_in = dram.tile([1, 1], f32, tag="ar0_in", name="ar0_in")
        ar0_out = dram.tile([1, 8], f32, tag="ar0_out", name="ar0_out")
        arm2_in = dram.tile([1, 1], f32, tag="arm2_in", name="arm2_in")
        arm2_out = dram.tile([1, 8], f32, tag="arm2_out", name="arm2_out")
        ar1_in = dram.tile([1, 2], f32, tag="ar1_in", name="ar1_in")
        ar1_out = dram.tile([1, 2], f32, tag="ar1_out", name="ar1_out")
        arv_in = dram.tile([1, 1], f32, tag="arv_in", name="arv_in")
        arv_out = dram.tile([1, 1], f32, tag="arv_out", name="arv_out")
        ar2_in = dram.tile([1, 1], f32, tag="ar2_in", name="ar2_in")
        ar2_out = dram.tile([1, 1], f32, tag="ar2_out", name="ar2_out")
        ar3_in = dram.tile([1, 1], f32, tag="ar3_in", name="ar3_in")
        ar3_out = dram.tile([1, 1], f32, tag="ar3_out", name="ar3_out")

        nc.vector.memset(magic_col[:], MAGIC)

        # per-output-channel qkv bias columns: one strided DMA, issued
        # before anything else queues on gpsimd
        nc.gpsimd.dma_start(
            bqs_cols[:, 0:12],
            bqs_ext[0:1, 0:1536].rearrange("a (t p) -> (a p) t", p=128),
        )

        # PE warm-up burst (ramp the PE clock) while DMAs fly
        wut = pp.tile([128, 512], bf16, tag="wut", name="wut")
        nc.vector.memset(wut[:], 1.0)
        with tc.tile_pool(name="pswu", bufs=1, space="PSUM") as pswu:
            wps = pswu.tile([128, 512], f32, tag="wps", name="wps")
            for _ in range(24):
                nc.tensor.matmul(wps[:], lhsT=wut[:, 0:128], rhs=wut[:], start=True, stop=True)
        nc.vector.memset(nmagic_col[:], -MAGIC)
        with tc.tile_pool(name="brow", bufs=1) as br:
            bvrow = br.tile([1, C], f32, tag="bvrow", name="bvrow")
            nc.sync.dma_start(bvrow[:], bqs_ext[0:1, 1536:2304])
            nc.gpsimd.partition_broadcast(bv_bc[:], bvrow[:])

        # late pool: tensors born mid-kernel (vq during phase A, bp_bc at proj)
        with tc.tile_pool(name="late", bufs=1) as lp:
         vq = [lp.tile([128, C], bf16, tag=f"vq{t}", name=f"vq{t}") for t in range(8)]
         bp_bc = lp.tile([128, C], f32, tag="bp_bc", name="bp_bc")
         # v_f persists until v-quant (on gpsimd during phase A)
         with tc.tile_pool(name="vf_pool", bufs=1) as vfp:
          v_f = [vfp.tile([128, C], f32, tag=f"vf{t}", name=f"vf{t}") for t in range(8)]

          # ---- stage 1+2: x quant, QKV matmuls, AG#1 (q,k), quantize -----
          with (
            tc.tile_pool(name="wload", bufs=1) as wl,
            tc.tile_pool(name="qkvf_pool", bufs=1) as qp,
            tc.tile_pool(name="s12", bufs=2) as s12,
            tc.tile_pool(name="psq", bufs=4, space="PSUM") as psq,
          ):
            wq_bf = [wl.tile([128, 3 * C], bf16, tag=f"wq{t}", name=f"wq{t}") for t in range(6)]
            xq = [wl.tile([128, N], bf16, tag=f"xq{t}", name=f"xq{t}") for t in range(6)]
            qkv_f = [qp.tile([128, N], f32, tag=f"qkvf{t}", name=f"qkvf{t}") for t in range(12)]

            # int8 inputs: DMA then upcast to bf16 (x-quant happened on host)
            for t in range(6):
                x8 = wl.tile([128, N], i8, tag="x8", name=f"x8{t}", bufs=2)
                wq8 = wl.tile([128, 3 * C], i8, tag="wq8", name=f"wq8{t}", bufs=2)
                nc.sync.dma_start(x8[:], xT_ext[t * 128 : (t + 1) * 128, :])
                nc.sync.dma_start(wq8[:], wq_qkv_ext[t * 128 : (t + 1) * 128, :])
                nc.gpsimd.tensor_copy(xq[t][:], x8[:])
                nc.vector.tensor_copy(wq_bf[t][:], wq8[:])

            # q/k part: per-mt accumulation over kt
            for mt in range(12):
                pss = [psq.tile([128, 512], f32, tag="psq", name="psq", bufs=6) for _ in range(2)]
                for kt in range(6):
                    for nk in range(2):
                        mi = nc.tensor.matmul(
                            pss[nk][:],
                            lhsT=wq_bf[kt][:, mt * 128 : (mt + 1) * 128],
                            rhs=xq[kt][:, nk * 512 : (nk + 1) * 512],
                            start=(kt == 0),
                            stop=(kt == 5),
                            skip_group_check=True,
                        )
                        if nk == 1:
                            mi.ins.ldweights = False
                for nk in range(2):
                    nc.scalar.activation(
                        qkv_f[mt][:, nk * 512 : (nk + 1) * 512],
                        pss[nk][:],
                        ACT.Identity,
                        bias=bqs_cols[:, mt : mt + 1],
                    )
                    nc.vector.tensor_reduce(
                        qkv_abs[:, mt * 2 + nk : mt * 2 + nk + 1],
                        qkv_f[mt][:, nk * 512 : (nk + 1) * 512],
                        axis=AX.X,
                        op=ALU.max,
                        apply_absolute_value=True,
                    )

            # ---- AG#1: global absmax of q, k --------------------------------
            am2 = s12.tile([128, 2], f32, tag="am2", name="am2")
            nc.vector.tensor_reduce(am2[:, 0:1], qkv_abs[:, 0:12], axis=AX.X, op=ALU.max)
            nc.vector.tensor_reduce(am2[:, 1:2], qkv_abs[:, 12:24], axis=AX.X, op=ALU.max)
            am2r = s12.tile([128, 2], f32, tag="am2r", name="am2r")
            nc.gpsimd.partition_all_reduce(am2r[:], am2[:], 128, RED.max)
            nc.gpsimd.dma_start(ar1_in[:], am2r[0:1, :])
            nc.gpsimd.collective_compute(
                "AllReduce", ALU.max, replica_groups=RG, ins=[ar1_in.opt()], outs=[ar1_out.opt()]
            )
            g2 = pp.tile([1, 2], f32, tag="g2", name="g2")
            nc.gpsimd.dma_start(g2[:], ar1_out[0:1, 0:2])
            g2b = pp.tile([128, 2], f32, tag="g2b", name="g2b")
            nc.gpsimd.partition_broadcast(g2b[:], g2[:])

            # aug zero/one fills: per-head chunks, emitted after the AG#1
            # fire so they don't clog the gpsimd queue ahead of it
            for h in range(H):
                nc.gpsimd.memset(aug_q[64:128, h * N : (h + 1) * N], 0.0)
                nc.gpsimd.memset(aug_k[64:128, h * N : (h + 1) * N], 0.0)
                nc.gpsimd.memset(aug_k[64:66, h * N : (h + 1) * N], 1.0)

            inv2 = pp.tile([128, 2], f32, tag="inv2", name="inv2")
            nc.vector.reciprocal(inv2[:], g2b[:])
            nc.vector.tensor_scalar(inv2[:], inv2[:], 127.0, None, ALU.mult)
            nc.vector.tensor_tensor(sc[:, 3:4], g2b[:, 0:1], g2b[:, 1:2], ALU.mult)
            nc.vector.tensor_scalar(sc[:, 3:4], sc[:, 3:4], CATT_CONST[0], None, ALU.mult)
            nc.vector.reciprocal(sc[:, 9:10], sc[:, 3:4])

            # v matmuls fill the PE idle window during/after AG#1 (their
            # absmax rides the separate AG#v)
            for nt in range(8):
                pss = []
                for ick, (ck, cw) in enumerate(((0, 512), (512, 256))):
                    pss.append((psq.tile([128, 512], f32, tag="psq", name="psv", bufs=6), ck, cw))
                for kt in range(6):
                    for ick2, (ps, ck, cw) in enumerate(pss):
                        mi = nc.tensor.matmul(
                            ps[:, 0:cw],
                            lhsT=xq[kt][:, nt * 128 : (nt + 1) * 128],
                            rhs=wq_bf[kt][:, 1536 + ck : 1536 + ck + cw],
                            start=(kt == 0),
                            stop=(kt == 5),
                            skip_group_check=True,
                        )
                        if ick2 == 1:
                            mi.ins.ldweights = False
                for ick, (ps, ck, cw) in enumerate(pss):
                    nc.vector.scalar_tensor_tensor(
                        v_f[nt][:, ck : ck + cw],
                        ps[:, 0:cw],
                        1.0,
                        bv_bc[:, ck : ck + cw],
                        ALU.mult,
                        ALU.add,
                    )
                    nc.vector.tensor_reduce(
                        v_abs[:, nt * 2 + ick : nt * 2 + ick + 1],
                        v_f[nt][:, ck : ck + cw],
                        axis=AX.X,
                        op=ALU.max,
                        apply_absolute_value=True,
                    )

            # ---- AG#v: global absmax of v (fire before phase A) -------------
            vam = s12.tile([128, 1], f32, tag="vam", name="vam")
            nc.vector.tensor_reduce(vam[:], v_abs[:], axis=AX.X, op=ALU.max)
            vamr = s12.tile([128, 1], f32, tag="vamr", name="vamr")
            nc.gpsimd.partition_all_reduce(vamr[:], vam[:], 128, RED.max)
            nc.gpsimd.dma_start(arv_in[:], vamr[0:1, :])
            nc.gpsimd.collective_compute(
                "AllReduce", ALU.max, replica_groups=RG, ins=[arv_in.opt()], outs=[arv_out.opt()]
            )
            gv = pp.tile([1, 1], f32, tag="gv", name="gv")
            nc.gpsimd.dma_start(gv[:], arv_out[0:1, 0:1])
            nc.gpsimd.partition_broadcast(sc[:, 14:15], gv[:])
            nc.vector.reciprocal(sc[:, 15:16], sc[:, 14:15])
            nc.vector.tensor_scalar(sc[:, 15:16], sc[:, 15:16], 127.0, None, ALU.mult)

            # ---- quantize q/k into the aug mega-tiles ----------------------
            for i, mt in enumerate((0, 6, 1, 7, 2, 8, 3, 9, 4, 10, 5, 11)):
                inv = inv2[:, 0:1] if mt < 6 else inv2[:, 1:2]
                y = s12.tile([128, N], f32, tag="s12y", name="yq", bufs=2)
                nc.scalar.activation(y[:], qkv_f[mt][:], ACT.Identity, bias=magic_col[:], scale=inv)
                qsc = s12.tile([128, N], bf16, tag="qsc", name="qsc", bufs=2)
                nc.vector.tensor_scalar(qsc[:], y[:], MAGIC, None, ALU.subtract)
                dst = aug_q if mt < 6 else aug_k
                tt = mt if mt < 6 else mt - 6
                nc.sync.dma_start(dst[0:64, (2 * tt) * N : (2 * tt + 1) * N], qsc[0:64, :])
                nc.sync.dma_start(dst[0:64, (2 * tt + 1) * N : (2 * tt + 2) * N], qsc[64:128, :])


          # ---- phase A: attn[i,j] logits stats -----------------------------
          with (
            tc.tile_pool(name="phA", bufs=4) as pa,
            tc.tile_pool(name="psA", bufs=3, space="PSUM") as psa,
          ):
            armid_dummy = pp.tile([1, 8], f32, tag="armid_dummy", name="armid_dummy")
            for h in range(H):
                if h == 6:
                    # resync collective: input depends on head-5 stats so the
                    # fire can't be hoisted before mid-phase-A; absorbs
                    # inter-core drift under the compute shadow
                    nc.gpsimd.dma_start(ar0_in[:], zbuf[0:1, 47:48])
                    nc.gpsimd.collective_compute(
                        "AllGather", ALU.bypass, replica_groups=RG,
                        ins=[ar0_in.opt()], outs=[ar0_out.opt()],
                    )
                if h == 9:
                    nc.gpsimd.dma_start(armid_dummy[:], ar0_out[0:1, :])
                for it in range(8):
                    psl = psa.tile([128, N], f32, tag="psl", name="psl")
                    for jc in range(2):
                        mi = nc.tensor.matmul(
                            psl[:, jc * 512 : (jc + 1) * 512],
                            lhsT=aug_q[:, h * N + it * 128 : h * N + (it + 1) * 128],
                            rhs=aug_k[:, h * N + jc * 512 : h * N + (jc + 1) * 512],
                            start=True,
                            stop=True,
                        )
                        if jc == 1:
                            mi.ins.ldweights = False
                    col = h * 8 + it
                    ea = pa.tile([128, N], bf16, tag="ea", name="ea")
                    nc.scalar.activation(
                        ea[:], psl[:], ACT.Exp, scale=sc[:, 3:4],
                        accum_out=zbuf[:, col : col + 1],
                    )
                    nc.vector.tensor_reduce(mlbuf[:, col : col + 1], psl[:], axis=AX.X, op=ALU.max)
                    wps2 = psa.tile([128, 512], f32, tag="wps2", name="wps2", bufs=2)
                    nc.tensor.matmul(wps2[:], lhsT=wut[:, 0:128], rhs=ea[:, 0:512], start=True, stop=True)

          # ---- AG#2: max prob; ln(r)/c rows --------------------------------
          with (
            tc.tile_pool(name="phR", bufs=1) as pr,
            tc.tile_pool(name="psT", bufs=1, space="PSUM") as pst,
          ):
            from concourse.masks import make_identity

            maxe = pr.tile([128, 96], f32, tag="maxe", name="maxe")
            nc.scalar.activation(maxe[:], mlbuf[:], ACT.Exp, scale=sc[:, 3:4])
            rz = pr.tile([128, 96], f32, tag="rz", name="rz")
            nc.vector.reciprocal(rz[:], zbuf[:])
            mp = pr.tile([128, 96], f32, tag="mp", name="mp")
            nc.vector.tensor_tensor(mp[:], maxe[:], rz[:], ALU.mult)
            pk1 = pr.tile([128, 1], f32, tag="pk1", name="pk1")
            nc.vector.tensor_reduce(pk1[:], mp[:], axis=AX.X, op=ALU.max)
            pk1r = pr.tile([128, 1], f32, tag="pk1r", name="pk1r")
            nc.gpsimd.partition_all_reduce(pk1r[:], pk1[:], 128, RED.max)
            nc.gpsimd.dma_start(ar2_in[:], pk1r[0:1, :])
            nc.gpsimd.collective_compute(
                "AllReduce", ALU.max, replica_groups=RG, ins=[ar2_in.opt()], outs=[ar2_out.opt()]
            )

            # -- PE keep-warm during AG#2 window ----------------------------
            wpsw = pst.tile([128, 512], f32, tag="wpsw", name="wpsw", bufs=2)
            wpsw2 = pst.tile([128, 512], f32, tag="wpsw", name="wpsw2", bufs=2)
            for wi in range(8):
                nc.tensor.matmul(
                    (wpsw if wi % 2 == 0 else wpsw2)[:],
                    lhsT=wut[:, 0:128], rhs=wut[:], start=True, stop=True,
                )

            # -- during AG#2 flight: v-quant (needs only AG#v result) --------
            for nt in range(8):
                yv = pr.tile([128, C], f32, tag="yv", name="yv", bufs=2)
                nc.scalar.activation(yv[:], v_f[nt][:], ACT.Identity, bias=magic_col[:], scale=sc[:, 15:16])
                nc.vector.tensor_scalar(vq[nt][:], yv[:], MAGIC, None, ALU.subtract)

            # -- during AG#2 flight: ln(r)/catt, transposed ------------------
            lnr = pr.tile([128, 96], f32, tag="lnr", name="lnr")
            nc.scalar.activation(lnr[:], rz[:], ACT.Ln)
            lnrc = pr.tile([128, 128], f32, tag="lnrc", name="lnrc")
            nc.vector.memset(lnrc[:], 0.0)
            nc.vector.tensor_scalar(lnrc[:, 0:96], lnr[:], sc[:, 9:10], None, ALU.mult)
            idn = pr.tile([128, 128], f32, tag="idn", name="idn")
            make_identity(nc, idn[:])
            psT = pst.tile([128, 128], f32, tag="psT", name="psT")
            nc.tensor.transpose(psT[:], lnrc[:], idn[:])
            lnrcT_pre = pr.tile([128, 128], f32, tag="lnrcT_pre", name="lnrcT_pre")
            nc.scalar.activation(lnrcT_pre[:], psT[:], ACT.Copy)

            # -- AG#2 result: maxp_g ----------------------------------------
            g2p = pr.tile([1, 1], f32, tag="g2p", name="g2p")
            nc.gpsimd.dma_start(g2p[:], ar2_out[0:1, 0:1])
            nc.gpsimd.partition_broadcast(sc[:, 7:8], g2p[:])
            nc.vector.reciprocal(sc[:, 8:9], sc[:, 7:8])
            nc.vector.tensor_scalar(sc[:, 8:9], sc[:, 8:9], 127.0, None, ALU.mult)
            # one strided DMA: [96,128] -> [12, 1024]; runs during AG#2 flight
            lnrc_rows = pr.tile([H, N], f32, tag="lnrc_rows", name="lnrc_rows")
            nc.scalar.dma_start(lnrc_rows[:], lnrcT_pre[0:96, 0:128])
            # cterm = ln(127/maxp)/catt folded into the hi/lo split directly
            cterm = pr.tile([128, 1], f32, tag="cterm", name="cterm")
            nc.scalar.activation(cterm[:], sc[:, 8:9], ACT.Ln)
            nc.vector.tensor_tensor(cterm[:], cterm[:], sc[:, 9:10], ALU.mult)
            nc.vector.tensor_scalar(hi_rows[:], lnrc_rows[:], cterm[0:H, 0:1], None, ALU.add)
            nc.vector.scalar_tensor_tensor(
                lo_rows[:], lnrc_rows[:], cterm[0:H, 0:1], hi_rows[:], ALU.add, ALU.subtract
            )
            # two strided DMAs land hi/lo into aug_q rows 64/65
            nc.scalar.dma_start(aug_q[64:65, :], hi_rows[:])
            nc.scalar.dma_start(aug_q[65:66, :], lo_rows[:])

         # ---- phase B: quantized probs + PV (zero-padded vz operands) ----
         with tc.tile_pool(name="oint_pool", bufs=1) as op_:
           o_int = [op_.tile([128, N], f32, tag=f"oint{t}", name=f"oint{t}") for t in range(6)]
           wp_bf = [op_.tile([128, C], bf16, tag=f"wp{t}", name=f"wp{t}") for t in range(6)]
           wp8 = [op_.tile([128, C], i8, tag=f"wp8{t}", name=f"wp8{t}") for t in range(6)]
           # prefetch proj weights + bias during phase B (gpsimd queue)
           for t in range(6):
               nc.gpsimd.dma_start(wp8[t][:], wq_proj_ext[t * 128 : (t + 1) * 128, :])
               nc.gpsimd.tensor_copy(wp_bf[t][:], wp8[t][:])
           with tc.tile_pool(name="brow2", bufs=1) as br2:
               bprow = br2.tile([1, C], f32, tag="bprow", name="bprow")
               nc.gpsimd.dma_start(bprow[:], bp_ext[:])
               nc.gpsimd.partition_broadcast(bp_bc[:], bprow[:])
           with (
             tc.tile_pool(name="phB", bufs=4) as pb,
             tc.tile_pool(name="vzp", bufs=2) as vzp,
             tc.tile_pool(name="psB", bufs=2, space="PSUM") as psb,
             tc.tile_pool(name="psO", bufs=2, space="PSUM") as pso_pool,
           ):
             armid2_dummy = pp.tile([1, 8], f32, tag="armid2_dummy", name="armid2_dummy")
             for hp in range(6):
                 if hp == 3:
                     nc.gpsimd.dma_start(arm2_in[:], o_abs[0:1, 2:3])
                     nc.gpsimd.collective_compute(
                         "AllGather", ALU.bypass, replica_groups=RG,
                         ins=[arm2_in.opt()], outs=[arm2_out.opt()],
                     )
                 if hp == 5:
                     nc.gpsimd.dma_start(armid2_dummy[:], arm2_out[0:1, :])
                 h0, h1 = 2 * hp, 2 * hp + 1
                 # padded PV weights: vz[:, jt*256 + 0:64] = v cols of h0,
                 # vz[:, jt*256 + 192:256] = v cols of h1, rest zero.
                 vz = vzp.tile([128, 8 * 256], bf16, tag="vz", name="vz")
                 nc.gpsimd.memset(vz[:], 0.0)
                 for jt in range(8):
                     nc.sync.dma_start(
                         vz[:, jt * 256 : jt * 256 + 64], vq[jt][:, h0 * 64 : (h0 + 1) * 64]
                     )
                     nc.sync.dma_start(
                         vz[:, jt * 256 + 192 : jt * 256 + 256], vq[jt][:, h1 * 64 : (h1 + 1) * 64]
                     )
                 pso = pso_pool.tile([128, N], f32, tag="pso", name="pso")
                 for jt in range(8):
                     pqs = []
                     for h in (h0, h1):
                         pslT = psb.tile([128, N], f32, tag="pslT", name="pslT")
                         for ic in range(2):
                             mi = nc.tensor.matmul(
                                 pslT[:, ic * 512 : (ic + 1) * 512],
                                 lhsT=aug_k[:, h * N + jt * 128 : h * N + (jt + 1) * 128],
                                 rhs=aug_q[:, h * N + ic * 512 : h * N + (ic + 1) * 512],
                                 start=True,
                                 stop=True,
                             )
                             if ic == 1:
                                 mi.ins.ldweights = False
                         ep = pb.tile([128, N], f32, tag="ep", name="ep")
                         nc.scalar.activation(ep[:], pslT[:], ACT.Exp, scale=sc[:, 3:4])
                         pq = pb.tile([128, N], bf16, tag="pq", name="pq")
                         nc.vector.tensor_scalar(pq[:], ep[:], MAGIC, MAGIC, ALU.add, ALU.subtract)
                         pqs.append(pq)
                     for hh, pq in enumerate(pqs):
                         for ic in range(2):
                             mi = nc.tensor.matmul(
                                 pso[:, ic * 512 : (ic + 1) * 512],
                                 lhsT=vz[:, jt * 256 + hh * 128 : jt * 256 + (hh + 1) * 128],
                                 rhs=pq[:, ic * 512 : (ic + 1) * 512],
                                 start=(jt == 0 and hh == 0),
                                 stop=(jt == 7 and hh == 1),
                                 skip_group_check=True,
                             )
                             if ic == 1:
                                 mi.ins.ldweights = False
                 nc.vector.tensor_copy(o_int[hp][:], pso[:])
                 nc.vector.tensor_reduce(
                     o_abs[:, hp : hp + 1], o_int[hp][:], axis=AX.X, op=ALU.max, apply_absolute_value=True
                 )

           # ---- AG#3 + quantize o + proj ----------------------------------
           with (
             tc.tile_pool(name="phC", bufs=3) as pc,
             tc.tile_pool(name="oq_pool", bufs=1) as oqp,
             tc.tile_pool(name="psF", bufs=4, space="PSUM") as psf_pool,
           ):
             oam = pc.tile([128, 1], f32, tag="oam", name="oam")
             nc.vector.tensor_reduce(oam[:], o_abs[:], axis=AX.X, op=ALU.max)
             oamr = pc.tile([128, 1], f32, tag="oamr", name="oamr")
             nc.gpsimd.partition_all_reduce(oamr[:], oam[:], 128, RED.max)
             nc.gpsimd.dma_start(ar3_in[:], oamr[0:1, :])
             nc.gpsimd.collective_compute(
                 "AllReduce", ALU.max, replica_groups=RG, ins=[ar3_in.opt()], outs=[ar3_out.opt()]
             )
             g3 = pc.tile([1, 1], f32, tag="g3", name="g3")
             nc.gpsimd.dma_start(g3[:], ar3_out[0:1, 0:1])
             nc.gpsimd.partition_broadcast(sc[:, 10:11], g3[:])

             nc.vector.reciprocal(sc[:, 11:12], sc[:, 10:11])
             nc.vector.tensor_scalar(sc[:, 11:12], sc[:, 11:12], 127.0, None, ALU.mult)
             nc.vector.tensor_tensor(sc[:, 12:13], sc[:, 7:8], sc[:, 14:15], ALU.mult)
             nc.vector.tensor_tensor(sc[:, 12:13], sc[:, 12:13], sc[:, 10:11], ALU.mult)
             nc.vector.tensor_scalar(sc[:, 12:13], sc[:, 12:13], SFIN_CONST[0], None, ALU.mult)

             oq = [oqp.tile([128, N], bf16, tag=f"oq{t}", name=f"oq{t}") for t in range(6)]
             for t in range(6):
                 if t % 2 == 0:
                     y = pc.tile([128, N], f32, tag="yo", name="yo")
                     nc.scalar.activation(y[:], o_int[t][:], ACT.Identity, bias=magic_col[:], scale=sc[:, 11:12])
                     nc.vector.tensor_scalar(oq[t][:], y[:], MAGIC, None, ALU.subtract)
                 else:
                     y = pc.tile([128, N], f32, tag="yo", name="yo")
                     nc.vector.tensor_scalar(y[:], o_int[t][:], sc[:, 11:12], MAGIC, ALU.mult, ALU.add)
                     nc.vector.tensor_scalar(oq[t][:], y[:], MAGIC, None, ALU.subtract)

             for g in range(2):
                 psfs = [psf_pool.tile([128, C], f32, tag="psf", name="psf") for _ in range(4)]
                 if g == 0:
                     for wi in range(10):
                         nc.tensor.matmul(
                             psfs[wi % 2][:, 0:512],
                             lhsT=wut[:, 0:128], rhs=wut[:], start=True, stop=True,
                         )
                 for kt in range(6):
                     for nn in range(4):
                         nt = g * 4 + nn
                         for ick2, (ck, cw) in enumerate(((0, 512), (512, 256))):
                             mi = nc.tensor.matmul(
                                 psfs[nn][:, ck : ck + cw],
                                 lhsT=oq[kt][:, nt * 128 : (nt + 1) * 128],
                                 rhs=wp_bf[kt][:, ck : ck + cw],
                                 start=(kt == 0),
                                 stop=(kt == 5),
                                 skip_group_check=True,
                             )
                             if ick2 == 1:
                                 mi.ins.ldweights = False
                 for nn in range(4):
                     nt = g * 4 + nn
                     ot = pc.tile([128, C], f32, tag="ot", name="ot")
                     nc.vector.scalar_tensor_tensor(
                         ot[:], psfs[nn][:], sc[:, 12:13], bp_bc[:], ALU.mult, ALU.add
                     )
                     nc.sync.dma_start(out_ext[nt * 128 : (nt + 1) * 128, :], ot[:])


def _host_prep(x, w_qkv, b_qkv, w_proj, b_proj):
    x = np.asarray(x, dtype=np.float32)
    w_qkv = np.asarray(w_qkv, dtype=np.float32)
    b_qkv = np.asarray(b_qkv, dtype=np.float32)
    w_proj = np.asarray(w_proj, dtype=np.float32)
    b_proj = np.asarray(b_proj, dtype=np.float32)

    qmax = np.float32(127.0)
    s_x = np.maximum(np.max(np.abs(x)) / qmax, np.float32(1e-8))
    s_wq = np.maximum(np.max(np.abs(w_qkv)) / qmax, np.float32(1e-8))
    s_wp = np.maximum(np.max(np.abs(w_proj)) / qmax, np.float32(1e-8))
    inv_s_x = float(np.float32(1.0) / s_x)

    wq_qkv = np.round(w_qkv / s_wq).astype(np.int8)
    wq_proj = np.round(w_proj / s_wp).astype(np.int8)
    bqs = (b_qkv / (s_x * s_wq)).astype(np.float32)[None, :]
    bp = b_proj.astype(np.float32)[None, :]

    sxw = float(s_x) * float(s_wq)
    sfin = float(s_wp) * sxw / (127.0**3)
    catt = 0.125 * sxw * sxw / (127.0 * 127.0)
    inv32 = np.float32(inv_s_x)
    in_maps = [
        {
            "xT": np.round(np.ascontiguousarray(x[b].T) * inv32).astype(np.int8),
            "wq_qkv": wq_qkv,
            "wq_proj": wq_proj,
            "bqs": bqs,
            "bp": bp,
        }
        for b in range(B)
    ]
    return inv_s_x, sfin, catt, in_maps


_CACHE = {}


def kernel(x, w_qkv, b_qkv, w_proj, b_proj):
    inv_s_x, sfin, catt, in_maps = _host_prep(x, w_qkv, b_qkv, w_proj, b_proj)
    key = (inv_s_x, sfin, catt)
    if key not in _CACHE:
        SFIN_CONST[0] = sfin
        CATT_CONST[0] = catt
        _CACHE[key] = build_graph(inv_s_x)
    nc = _CACHE[key]
    res = run_bass_kernel_spmd(nc, in_maps, CORES)
    return np.stack([res.results[b]["out"] for b in range(B)], axis=0)


def build_and_inmaps(x, w_qkv, b_qkv, w_proj, b_proj):
    inv_s_x, sfin, catt, in_maps = _host_prep(x, w_qkv, b_qkv, w_proj, b_proj)
    SFIN_CONST[0] = sfin
    CATT_CONST[0] = catt
    nc = build_graph(inv_s_x)
    return nc, in_maps


# revision 14
# speedup vs baseline: 1.4693x; 1.0668x over previous
"""Fake-quantized multi-head attention block on 8 TRN2 NeuronCores.

Data-parallel over batch (1 element per core); integer-domain quantized
matmuls in bf16; global fake-quant scales via tiny AllGather
collectives + local max.  Key structural points:

- A dummy AllGather fires at t=0 to absorb SPMD launch skew while
  stage-1 runs; later collectives then cost ~2-3us instead of ~12-27us.
- aug mega-tensors [128, 12*N]: rows 0:64 = q/k head slices (ints),
  aug_k rows 64:66 = ones, aug_q rows 64:66 = hi/lo split of
  ln(r*127/maxp)/catt written after AG#2, rows 66:128 zero.  Full
  128-row operands in BOTH attention phases keep the PE HAM clock-gate
  at full speed (64-row operands measurably halve the PE clock).
- v absmax rides its own early AllGather (AG#v) fired before phase A;
  v-quantize runs on gpsimd during phase A (gpsimd is otherwise idle).
- ln(r)/catt is computed and PE-transposed during AG#2 flight; the
  global -ln(maxp/127)/catt constant lands as one ACT bias add, and the
  hi/lo rows land in the aug tile with 2 strided DMAs.
"""

import sys

sys.path.insert(0, "/opt/trn_rl_repo")

import numpy as np
import ml_dtypes

import concourse.mybir as mybir
import concourse.tile as tile
import concourse.bass_isa as bass_isa
from concourse import bacc
from concourse.bass_utils import run_bass_kernel_spmd

f32 = mybir.dt.float32
bf16 = mybir.dt.bfloat16
i8 = mybir.dt.int8
ALU = mybir.AluOpType
ACT = mybir.ActivationFunctionType
AX = mybir.AxisListType
RED = bass_isa.ReduceOp

B, N, C = 8, 1024, 768
H, HD = 12, 64
NCORES = 8
MAGIC = float(np.float32(3 * 2**22))
CORES = list(range(NCORES))
RG = [CORES]

SFIN_CONST = [1.0]
CATT_CONST = [1.0]


def build_graph(inv_s_x: float):
    nc = bacc.Bacc("TRN2", target_bir_lowering=False, debug=False, num_devices=NCORES)

    xT_ext = nc.dram_tensor("xT", [C, N], i8, kind="ExternalInput")
    wq_qkv_ext = nc.dram_tensor("wq_qkv", [C, 3 * C], i8, kind="ExternalInput")
    wq_proj_ext = nc.dram_tensor("wq_proj", [C, C], bf16, kind="ExternalInput")
    bqs_ext = nc.dram_tensor("bqs", [1, 3 * C], f32, kind="ExternalInput")
    bp_ext = nc.dram_tensor("bp", [1, C], f32, kind="ExternalInput")
    out_ext = nc.dram_tensor("out", [N, C], f32, kind="ExternalOutput")

    with tile.TileContext(nc) as tc:
        run_body(nc, tc, inv_s_x, xT_ext, wq_qkv_ext, wq_proj_ext, bqs_ext, bp_ext, out_ext)
    nc.finalize()
    return nc


def run_body(nc, tc, inv_s_x, xT_ext, wq_qkv_ext, wq_proj_ext, bqs_ext, bp_ext, out_ext):
    with (
        tc.tile_pool(name="persist", bufs=1) as pp,
        tc.tile_pool(name="dram", bufs=1, space="DRAM") as dram,
    ):
        # aug mega-tensors: column block h*N..(h+1)*N is head h
        aug_q = pp.tile([128, H * N], bf16, tag="aug_q", name="aug_q")
        aug_k = pp.tile([128, H * N], bf16, tag="aug_k", name="aug_k")
        zbuf = pp.tile([128, 96], f32, tag="zbuf", name="zbuf")
        mlbuf = pp.tile([128, 96], f32, tag="mlbuf", name="mlbuf")
        qkv_abs = pp.tile([128, 24], f32, tag="qkv_abs", name="qkv_abs")
        v_abs = pp.tile([128, 16], f32, tag="v_abs", name="v_abs")
        o_abs = pp.tile([128, 6], f32, tag="o_abs", name="o_abs")
        sc = pp.tile([128, 16], f32, tag="sc", name="sc")
        bqs_cols = pp.tile([128, 12], f32, tag="bqs_cols", name="bqs_cols")
        bv_bc = pp.tile([128, C], f32, tag="bv_bc", name="bv_bc")
        hi_rows = pp.tile([H, N], bf16, tag="hi_rows", name="hi_rows")
        lo_rows = pp.tile([H, N], bf16, tag="lo_rows", name="lo_rows")
        magic_col = pp.tile([128, 1], f32, tag="magic_col", name="magic_col")
        nmagic_col = pp.tile([128, 1], f32, tag="nmagic_col", name="nmagic_col")

        ar0_in = dram.tile([1, 1], f32, tag="ar0_in", name="ar0_in")
        ar0_out = dram.tile([1, 8], f32, tag="ar0_out", name="ar0_out")
        arm2_in = dram.tile([1, 1], f32, tag="arm2_in", name="arm2_in")
        arm2_out = dram.tile([1, 8], f32, tag="arm2_out", name="arm2_out")
        ar1_in = dram.tile([1, 2], f32, tag="ar1_in", name="ar1_in")
        ar1_out = dram.tile([1, 2], f32, tag="ar1_out", name="ar1_out")
        arv_in = dram.tile([1, 1], f32, tag="arv_in", name="arv_in")
        arv_out = dram.tile([1, 1], f32, tag="arv_out", name="arv_out")
        ar2_in = dram.tile([1, 1], f32, tag="ar2_in", name="ar2_in")
        ar2_out = dram.tile([1, 1], f32, tag="ar2_out", name="ar2_out")
        ar3_in = dram.tile([1, 1], f32, tag="ar3_in", name="ar3_in")
        ar3_out = dram.tile([1, 1], f32, tag="ar3_out", name="ar3_out")

        nc.vector.memset(magic_col[:], MAGIC)

        # per-output-channel qkv bias columns: one strided DMA, issued
        # before anything else queues on gpsimd
        nc.gpsimd.dma_start(
            bqs_cols[:, 0:12],
            bqs_ext[0:1, 0:1536].rearrange("a (t p) -> (a p) t", p=128),
        )

        # PE warm-up burst (ramp the PE clock) while DMAs fly
        wut = pp.tile([128, 512], bf16, tag="wut", name="wut")
        nc.vector.memset(wut[:], 1.0)
        with tc.tile_pool(name="pswu", bufs=1, space="PSUM") as pswu:
            wps = pswu.tile([128, 512], f32, tag="wps", name="wps")
            for _ in range(24):
                nc.tensor.matmul(wps[:], lhsT=wut[:, 0:128], rhs=wut[:], start=True, stop=True)
        nc.vector.memset(nmagic_col[:], -MAGIC)
        with tc.tile_pool(name="brow", bufs=1) as br:
            bvrow = br.tile([1, C], f32, tag="bvrow", name="bvrow")
            nc.sync.dma_start(bvrow[:], bqs_ext[0:1, 1536:2304])
            nc.gpsimd.partition_broadcast(bv_bc[:], bvrow[:])

        # late pool: tensors born mid-kernel (vq during phase A, bp_bc at proj)
        with tc.tile_pool(name="late", bufs=1) as lp:
         vq = [lp.tile([128, C], bf16, tag=f"vq{t}", name=f"vq{t}") for t in range(8)]
         bp_bc = lp.tile([128, C], f32, tag="bp_bc", name="bp_bc")
         # v_f persists until v-quant (on gpsimd during phase A)
         with tc.tile_pool(name="vf_pool", bufs=1) as vfp:
          v_f = [vfp.tile([128, C], f32, tag=f"vf{t}", name=f"vf{t}") for t in range(8)]

          # ---- stage 1+2: x quant, QKV matmuls, AG#1 (q,k), quantize -----
          with (
            tc.tile_pool(name="wload", bufs=1) as wl,
            tc.tile_pool(name="qkvf_pool", bufs=1) as qp,
            tc.tile_pool(name="s12", bufs=2) as s12,
            tc.tile_pool(name="psq", bufs=4, space="PSUM") as psq,
          ):
            wq_bf = [wl.tile([128, 3 * C], bf16, tag=f"wq{t}", name=f"wq{t}") for t in range(6)]
            xq = [wl.tile([128, N], bf16, tag=f"xq{t}", name=f"xq{t}") for t in range(6)]
            qkv_f = [qp.tile([128, N], f32, tag=f"qkvf{t}", name=f"qkvf{t}") for t in range(12)]

            # int8 inputs: DMA then upcast to bf16 (x-quant happened on host)
            for t in range(6):
                x8 = wl.tile([128, N], i8, tag="x8", name=f"x8{t}", bufs=2)
                wq8 = wl.tile([128, 3 * C], i8, tag="wq8", name=f"wq8{t}", bufs=2)
                nc.sync.dma_start(x8[:], xT_ext[t * 128 : (t + 1) * 128, :])
                nc.sync.dma_start(wq8[:], wq_qkv_ext[t * 128 : (t + 1) * 128, :])
                nc.vector.tensor_copy(xq[t][:], x8[:])
                nc.vector.tensor_copy(wq_bf[t][:], wq8[:])

            # q/k part: per-mt accumulation over kt
            for mt in range(12):
                pss = [psq.tile([128, 512], f32, tag="psq", name="psq", bufs=6) for _ in range(2)]
                for kt in range(6):
                    for nk in range(2):
                        mi = nc.tensor.matmul(
                            pss[nk][:],
                            lhsT=wq_bf[kt][:, mt * 128 : (mt + 1) * 128],
                            rhs=xq[kt][:, nk * 512 : (nk + 1) * 512],
                            start=(kt == 0),
                            stop=(kt == 5),
                            skip_group_check=True,
                        )
                        if nk == 1:
                            mi.ins.ldweights = False
                for nk in range(2):
                    nc.scalar.activation(
                        qkv_f[mt][:, nk * 512 : (nk + 1) * 512],
                        pss[nk][:],
                        ACT.Identity,
                        bias=bqs_cols[:, mt : mt + 1],
                    )
                    nc.vector.tensor_reduce(
                        qkv_abs[:, mt * 2 + nk : mt * 2 + nk + 1],
                        qkv_f[mt][:, nk * 512 : (nk + 1) * 512],
                        axis=AX.X,
                        op=ALU.max,
                        apply_absolute_value=True,
                    )

            # ---- AG#1: global absmax of q, k --------------------------------
            am2 = s12.tile([128, 2], f32, tag="am2", name="am2")
            nc.vector.tensor_reduce(am2[:, 0:1], qkv_abs[:, 0:12], axis=AX.X, op=ALU.max)
            nc.vector.tensor_reduce(am2[:, 1:2], qkv_abs[:, 12:24], axis=AX.X, op=ALU.max)
            am2r = s12.tile([128, 2], f32, tag="am2r", name="am2r")
            nc.gpsimd.partition_all_reduce(am2r[:], am2[:], 128, RED.max)
            nc.gpsimd.dma_start(ar1_in[:], am2r[0:1, :])
            nc.gpsimd.collective_compute(
                "AllReduce", ALU.max, replica_groups=RG, ins=[ar1_in.opt()], outs=[ar1_out.opt()]
            )
            g2 = pp.tile([1, 2], f32, tag="g2", name="g2")
            nc.gpsimd.dma_start(g2[:], ar1_out[0:1, 0:2])
            g2b = pp.tile([128, 2], f32, tag="g2b", name="g2b")
            nc.gpsimd.partition_broadcast(g2b[:], g2[:])

            # aug zero/one fills: per-head chunks, emitted after the AG#1
            # fire so they don't clog the gpsimd queue ahead of it
            for h in range(H):
                nc.gpsimd.memset(aug_q[64:128, h * N : (h + 1) * N], 0.0)
                nc.gpsimd.memset(aug_k[64:128, h * N : (h + 1) * N], 0.0)
                nc.gpsimd.memset(aug_k[64:66, h * N : (h + 1) * N], 1.0)

            inv2 = pp.tile([128, 2], f32, tag="inv2", name="inv2")
            nc.vector.reciprocal(inv2[:], g2b[:])
            nc.vector.tensor_scalar(inv2[:], inv2[:], 127.0, None, ALU.mult)
            nc.vector.tensor_tensor(sc[:, 3:4], g2b[:, 0:1], g2b[:, 1:2], ALU.mult)
            nc.vector.tensor_scalar(sc[:, 3:4], sc[:, 3:4], CATT_CONST[0], None, ALU.mult)
            nc.vector.reciprocal(sc[:, 9:10], sc[:, 3:4])

            # v matmuls fill the PE idle window during/after AG#1 (their
            # absmax rides the separate AG#v)
            for nt in range(8):
                pss = []
                for ick, (ck, cw) in enumerate(((0, 512), (512, 256))):
                    pss.append((psq.tile([128, 512], f32, tag="psq", name="psv", bufs=6), ck, cw))
                for kt in range(6):
                    for ick2, (ps, ck, cw) in enumerate(pss):
                        mi = nc.tensor.matmul(
                            ps[:, 0:cw],
                            lhsT=xq[kt][:, nt * 128 : (nt + 1) * 128],
                            rhs=wq_bf[kt][:, 1536 + ck : 1536 + ck + cw],
                            start=(kt == 0),
                            stop=(kt == 5),
                            skip_group_check=True,
                        )
                        if ick2 == 1:
                            mi.ins.ldweights = False
                for ick, (ps, ck, cw) in enumerate(pss):
                    nc.vector.scalar_tensor_tensor(
                        v_f[nt][:, ck : ck + cw],
                        ps[:, 0:cw],
                        1.0,
                        bv_bc[:, ck : ck + cw],
                        ALU.mult,
                        ALU.add,
                    )
                    nc.vector.tensor_reduce(
                        v_abs[:, nt * 2 + ick : nt * 2 + ick + 1],
                        v_f[nt][:, ck : ck + cw],
                        axis=AX.X,
                        op=ALU.max,
                        apply_absolute_value=True,
                    )

            # ---- AG#v: global absmax of v (fire before phase A) -------------
            vam = s12.tile([128, 1], f32, tag="vam", name="vam")
            nc.vector.tensor_reduce(vam[:], v_abs[:], axis=AX.X, op=ALU.max)
            vamr = s12.tile([128, 1], f32, tag="vamr", name="vamr")
            nc.gpsimd.partition_all_reduce(vamr[:], vam[:], 128, RED.max)
            nc.gpsimd.dma_start(arv_in[:], vamr[0:1, :])
            nc.gpsimd.collective_compute(
                "AllReduce", ALU.max, replica_groups=RG, ins=[arv_in.opt()], outs=[arv_out.opt()]
            )
            gv = pp.tile([1, 1], f32, tag="gv", name="gv")
            nc.gpsimd.dma_start(gv[:], arv_out[0:1, 0:1])
            nc.gpsimd.partition_broadcast(sc[:, 14:15], gv[:])
            nc.vector.reciprocal(sc[:, 15:16], sc[:, 14:15])
            nc.vector.tensor_scalar(sc[:, 15:16], sc[:, 15:16], 127.0, None, ALU.mult)

            # ---- quantize q/k into the aug mega-tiles ----------------------
            for i, mt in enumerate((0, 6, 1, 7, 2, 8, 3, 9, 4, 10, 5, 11)):
                inv = inv2[:, 0:1] if mt < 6 else inv2[:, 1:2]
                y = s12.tile([128, N], f32, tag="s12y", name="yq", bufs=2)
                nc.scalar.activation(y[:], qkv_f[mt][:], ACT.Identity, bias=magic_col[:], scale=inv)
                qsc = s12.tile([128, N], bf16, tag="qsc", name="qsc", bufs=2)
                nc.vector.tensor_scalar(qsc[:], y[:], MAGIC, None, ALU.subtract)
                dst = aug_q if mt < 6 else aug_k
                tt = mt if mt < 6 else mt - 6
                nc.sync.dma_start(dst[0:64, (2 * tt) * N : (2 * tt + 1) * N], qsc[0:64, :])
                nc.sync.dma_start(dst[0:64, (2 * tt + 1) * N : (2 * tt + 2) * N], qsc[64:128, :])


          # ---- phase A: attn[i,j] logits stats -----------------------------
          with (
            tc.tile_pool(name="phA", bufs=4) as pa,
            tc.tile_pool(name="psA", bufs=3, space="PSUM") as psa,
          ):
            armid_dummy = pp.tile([1, 8], f32, tag="armid_dummy", name="armid_dummy")
            for h in range(H):
                if h == 6:
                    # resync collective: input depends on head-5 stats so the
                    # fire can't be hoisted before mid-phase-A; absorbs
                    # inter-core drift under the compute shadow
                    nc.gpsimd.dma_start(ar0_in[:], zbuf[0:1, 47:48])
                    nc.gpsimd.collective_compute(
                        "AllGather", ALU.bypass, replica_groups=RG,
                        ins=[ar0_in.opt()], outs=[ar0_out.opt()],
                    )
                if h == 9:
                    nc.gpsimd.dma_start(armid_dummy[:], ar0_out[0:1, :])
                for it in range(8):
                    psl = psa.tile([128, N], f32, tag="psl", name="psl")
                    for jc in range(2):
                        mi = nc.tensor.matmul(
                            psl[:, jc * 512 : (jc + 1) * 512],
                            lhsT=aug_q[:, h * N + it * 128 : h * N + (it + 1) * 128],
                            rhs=aug_k[:, h * N + jc * 512 : h * N + (jc + 1) * 512],
                            start=True,
                            stop=True,
                        )
                        if jc == 1:
                            mi.ins.ldweights = False
                    col = h * 8 + it
                    ea = pa.tile([128, N], bf16, tag="ea", name="ea")
                    nc.scalar.activation(
                        ea[:], psl[:], ACT.Exp, scale=sc[:, 3:4],
                        accum_out=zbuf[:, col : col + 1],
                    )
                    nc.vector.tensor_reduce(mlbuf[:, col : col + 1], psl[:], axis=AX.X, op=ALU.max)
                    wps2 = psa.tile([128, 512], f32, tag="wps2", name="wps2", bufs=2)
                    nc.tensor.matmul(wps2[:], lhsT=wut[:, 0:128], rhs=ea[:, 0:512], start=True, stop=True)

          # ---- AG#2: max prob; ln(r)/c rows --------------------------------
          with (
            tc.tile_pool(name="phR", bufs=1) as pr,
            tc.tile_pool(name="psT", bufs=1, space="PSUM") as pst,
          ):
            from concourse.masks import make_identity

            maxe = pr.tile([128, 96], f32, tag="maxe", name="maxe")
            nc.scalar.activation(maxe[:], mlbuf[:], ACT.Exp, scale=sc[:, 3:4])
            rz = pr.tile([128, 96], f32, tag="rz", name="rz")
            nc.vector.reciprocal(rz[:], zbuf[:])
            mp = pr.tile([128, 96], f32, tag="mp", name="mp")
            nc.vector.tensor_tensor(mp[:], maxe[:], rz[:], ALU.mult)
            pk1 = pr.tile([128, 1], f32, tag="pk1", name="pk1")
            nc.vector.tensor_reduce(pk1[:], mp[:], axis=AX.X, op=ALU.max)
            pk1r = pr.tile([128, 1], f32, tag="pk1r", name="pk1r")
            nc.gpsimd.partition_all_reduce(pk1r[:], pk1[:], 128, RED.max)
            nc.gpsimd.dma_start(ar2_in[:], pk1r[0:1, :])
            nc.gpsimd.collective_compute(
                "AllReduce", ALU.max, replica_groups=RG, ins=[ar2_in.opt()], outs=[ar2_out.opt()]
            )

            # -- PE keep-warm during AG#2 window ----------------------------
            wpsw = pst.tile([128, 512], f32, tag="wpsw", name="wpsw", bufs=2)
            wpsw2 = pst.tile([128, 512], f32, tag="wpsw", name="wpsw2", bufs=2)
            for wi in range(8):
                nc.tensor.matmul(
                    (wpsw if wi % 2 == 0 else wpsw2)[:],
                    lhsT=wut[:, 0:128], rhs=wut[:], start=True, stop=True,
                )

            # -- during AG#2 flight: v-quant (needs only AG#v result) --------
            for nt in range(8):
                yv = pr.tile([128, C], f32, tag="yv", name="yv", bufs=2)
                nc.scalar.activation(yv[:], v_f[nt][:], ACT.Identity, bias=magic_col[:], scale=sc[:, 15:16])
                nc.vector.tensor_scalar(vq[nt][:], yv[:], MAGIC, None, ALU.subtract)

            # -- during AG#2 flight: ln(r)/catt, transposed ------------------
            lnr = pr.tile([128, 96], f32, tag="lnr", name="lnr")
            nc.scalar.activation(lnr[:], rz[:], ACT.Ln)
            lnrc = pr.tile([128, 128], f32, tag="lnrc", name="lnrc")
            nc.vector.memset(lnrc[:], 0.0)
            nc.vector.tensor_scalar(lnrc[:, 0:96], lnr[:], sc[:, 9:10], None, ALU.mult)
            idn = pr.tile([128, 128], f32, tag="idn", name="idn")
            make_identity(nc, idn[:])
            psT = pst.tile([128, 128], f32, tag="psT", name="psT")
            nc.tensor.transpose(psT[:], lnrc[:], idn[:])
            lnrcT_pre = pr.tile([128, 128], f32, tag="lnrcT_pre", name="lnrcT_pre")
            nc.scalar.activation(lnrcT_pre[:], psT[:], ACT.Copy)

            # -- AG#2 result: maxp_g ----------------------------------------
            g2p = pr.tile([1, 1], f32, tag="g2p", name="g2p")
            nc.gpsimd.dma_start(g2p[:], ar2_out[0:1, 0:1])
            nc.gpsimd.partition_broadcast(sc[:, 7:8], g2p[:])
            nc.vector.reciprocal(sc[:, 8:9], sc[:, 7:8])
            nc.vector.tensor_scalar(sc[:, 8:9], sc[:, 8:9], 127.0, None, ALU.mult)
            # one strided DMA: [96,128] -> [12, 1024]; runs during AG#2 flight
            lnrc_rows = pr.tile([H, N], f32, tag="lnrc_rows", name="lnrc_rows")
            nc.scalar.dma_start(lnrc_rows[:], lnrcT_pre[0:96, 0:128])
            # cterm = ln(127/maxp)/catt folded into the hi/lo split directly
            cterm = pr.tile([128, 1], f32, tag="cterm", name="cterm")
            nc.scalar.activation(cterm[:], sc[:, 8:9], ACT.Ln)
            nc.vector.tensor_tensor(cterm[:], cterm[:], sc[:, 9:10], ALU.mult)
            nc.vector.tensor_scalar(hi_rows[:], lnrc_rows[:], cterm[0:H, 0:1], None, ALU.add)
            nc.vector.scalar_tensor_tensor(
                lo_rows[:], lnrc_rows[:], cterm[0:H, 0:1], hi_rows[:], ALU.add, ALU.subtract
            )
            # two strided DMAs land hi/lo into aug_q rows 64/65
            nc.scalar.dma_start(aug_q[64:65, :], hi_rows[:])
            nc.scalar.dma_start(aug_q[65:66, :], lo_rows[:])

         # ---- phase B: quantized probs + PV (zero-padded vz operands) ----
         with tc.tile_pool(name="oint_pool", bufs=1) as op_:
           o_int = [op_.tile([128, N], f32, tag=f"oint{t}", name=f"oint{t}") for t in range(6)]
           wp_bf = [op_.tile([128, C], bf16, tag=f"wp{t}", name=f"wp{t}") for t in range(6)]
           # prefetch proj weights + bias during phase B (gpsimd queue)
           for t in range(6):
               nc.gpsimd.dma_start(wp_bf[t][:], wq_proj_ext[t * 128 : (t + 1) * 128, :])
           with tc.tile_pool(name="brow2", bufs=1) as br2:
               bprow = br2.tile([1, C], f32, tag="bprow", name="bprow")
               nc.gpsimd.dma_start(bprow[:], bp_ext[:])
               nc.gpsimd.partition_broadcast(bp_bc[:], bprow[:])
           with (
             tc.tile_pool(name="phB", bufs=4) as pb,
             tc.tile_pool(name="vzp", bufs=2) as vzp,
             tc.tile_pool(name="psB", bufs=2, space="PSUM") as psb,
             tc.tile_pool(name="psO", bufs=2, space="PSUM") as pso_pool,
           ):
             armid2_dummy = pp.tile([1, 8], f32, tag="armid2_dummy", name="armid2_dummy")
             for hp in range(6):
                 if hp == 3:
                     nc.gpsimd.dma_start(arm2_in[:], o_abs[0:1, 2:3])
                     nc.gpsimd.collective_compute(
                         "AllGather", ALU.bypass, replica_groups=RG,
                         ins=[arm2_in.opt()], outs=[arm2_out.opt()],
                     )
                 if hp == 5:
                     nc.gpsimd.dma_start(armid2_dummy[:], arm2_out[0:1, :])
                 h0, h1 = 2 * hp, 2 * hp + 1
                 # padded PV weights: vz[:, jt*256 + 0:64] = v cols of h0,
                 # vz[:, jt*256 + 192:256] = v cols of h1, rest zero.
                 vz = vzp.tile([128, 8 * 256], bf16, tag="vz", name="vz")
                 nc.gpsimd.memset(vz[:], 0.0)
                 for jt in range(8):
                     nc.sync.dma_start(
                         vz[:, jt * 256 : jt * 256 + 64], vq[jt][:, h0 * 64 : (h0 + 1) * 64]
                     )
                     nc.sync.dma_start(
                         vz[:, jt * 256 + 192 : jt * 256 + 256], vq[jt][:, h1 * 64 : (h1 + 1) * 64]
                     )
                 pso = pso_pool.tile([128, N], f32, tag="pso", name="pso")
                 for jt in range(8):
                     pqs = []
                     for h in (h0, h1):
                         pslT = psb.tile([128, N], f32, tag="pslT", name="pslT")
                         for ic in range(2):
                             mi = nc.tensor.matmul(
                                 pslT[:, ic * 512 : (ic + 1) * 512],
                                 lhsT=aug_k[:, h * N + jt * 128 : h * N + (jt + 1) * 128],
                                 rhs=aug_q[:, h * N + ic * 512 : h * N + (ic + 1) * 512],
                                 start=True,
                                 stop=True,
                             )
                             if ic == 1:
                                 mi.ins.ldweights = False
                         ep = pb.tile([128, N], f32, tag="ep", name="ep")
                         nc.scalar.activation(ep[:], pslT[:], ACT.Exp, scale=sc[:, 3:4])
                         pq = pb.tile([128, N], bf16, tag="pq", name="pq")
                         nc.vector.tensor_scalar(pq[:], ep[:], MAGIC, MAGIC, ALU.add, ALU.subtract)
                         pqs.append(pq)
                     for hh, pq in enumerate(pqs):
                         for ic in range(2):
                             mi = nc.tensor.matmul(
                                 pso[:, ic * 512 : (ic + 1) * 512],
                                 lhsT=vz[:, jt * 256 + hh * 128 : jt * 256 + (hh + 1) * 128],
                                 rhs=pq[:, ic * 512 : (ic + 1) * 512],
                                 start=(jt == 0 and hh == 0),
                                 stop=(jt == 7 and hh == 1),
                                 skip_group_check=True,
                             )
                             if ic == 1:
                                 mi.ins.ldweights = False
                 nc.vector.tensor_copy(o_int[hp][:], pso[:])
                 nc.vector.tensor_reduce(
                     o_abs[:, hp : hp + 1], o_int[hp][:], axis=AX.X, op=ALU.max, apply_absolute_value=True
                 )

           # ---- AG#3 + quantize o + proj ----------------------------------
           with (
             tc.tile_pool(name="phC", bufs=3) as pc,
             tc.tile_pool(name="oq_pool", bufs=1) as oqp,
             tc.tile_pool(name="psF", bufs=4, space="PSUM") as psf_pool,
           ):
             oam = pc.tile([128, 1], f32, tag="oam", name="oam")
             nc.vector.tensor_reduce(oam[:], o_abs[:], axis=AX.X, op=ALU.max)
             oamr = pc.tile([128, 1], f32, tag="oamr", name="oamr")
             nc.gpsimd.partition_all_reduce(oamr[:], oam[:], 128, RED.max)
             nc.gpsimd.dma_start(ar3_in[:], oamr[0:1, :])
             nc.gpsimd.collective_compute(
                 "AllReduce", ALU.max, replica_groups=RG, ins=[ar3_in.opt()], outs=[ar3_out.opt()]
             )
             g3 = pc.tile([1, 1], f32, tag="g3", name="g3")
             nc.gpsimd.dma_start(g3[:], ar3_out[0:1, 0:1])
             nc.gpsimd.partition_broadcast(sc[:, 10:11], g3[:])

             nc.vector.reciprocal(sc[:, 11:12], sc[:, 10:11])
             nc.vector.tensor_scalar(sc[:, 11:12], sc[:, 11:12], 127.0, None, ALU.mult)
             nc.vector.tensor_tensor(sc[:, 12:13], sc[:, 7:8], sc[:, 14:15], ALU.mult)
             nc.vector.tensor_tensor(sc[:, 12:13], sc[:, 12:13], sc[:, 10:11], ALU.mult)
             nc.vector.tensor_scalar(sc[:, 12:13], sc[:, 12:13], SFIN_CONST[0], None, ALU.mult)

             oq = [oqp.tile([128, N], bf16, tag=f"oq{t}", name=f"oq{t}") for t in range(6)]
             for t in range(6):
                 if t % 2 == 0:
                     y = pc.tile([128, N], f32, tag="yo", name="yo")
                     nc.scalar.activation(y[:], o_int[t][:], ACT.Identity, bias=magic_col[:], scale=sc[:, 11:12])
                     nc.vector.tensor_scalar(oq[t][:], y[:], MAGIC, None, ALU.subtract)
                 else:
                     y = pc.tile([128, N], f32, tag="yo", name="yo")
                     nc.vector.tensor_scalar(y[:], o_int[t][:], sc[:, 11:12], MAGIC, ALU.mult, ALU.add)
                     nc.vector.tensor_scalar(oq[t][:], y[:], MAGIC, None, ALU.subtract)

             for g in range(2):
                 psfs = [psf_pool.tile([128, C], f32, tag="psf", name="psf") for _ in range(4)]
                 if g == 0:
                     for wi in range(10):
                         nc.tensor.matmul(
                             psfs[wi % 2][:, 0:512],
                             lhsT=wut[:, 0:128], rhs=wut[:], start=True, stop=True,
                         )
                 for kt in range(6):
                     for nn in range(4):
                         nt = g * 4 + nn
                         for ick2, (ck, cw) in enumerate(((0, 512), (512, 256))):
                             mi = nc.tensor.matmul(
                                 psfs[nn][:, ck : ck + cw],
                                 lhsT=oq[kt][:, nt * 128 : (nt + 1) * 128],
                                 rhs=wp_bf[kt][:, ck : ck + cw],
                                 start=(kt == 0),
                                 stop=(kt == 5),
                                 skip_group_check=True,
                             )
                             if ick2 == 1:
                                 mi.ins.ldweights = False
                 for nn in range(4):
                     nt = g * 4 + nn
                     ot = pc.tile([128, C], f32, tag="ot", name="ot")
                     nc.vector.scalar_tensor_tensor(
                         ot[:], psfs[nn][:], sc[:, 12:13], bp_bc[:], ALU.mult, ALU.add
                     )
                     nc.sync.dma_start(out_ext[nt * 128 : (nt + 1) * 128, :], ot[:])


def _host_prep(x, w_qkv, b_qkv, w_proj, b_proj):
    x = np.asarray(x, dtype=np.float32)
    w_qkv = np.asarray(w_qkv, dtype=np.float32)
    b_qkv = np.asarray(b_qkv, dtype=np.float32)
    w_proj = np.asarray(w_proj, dtype=np.float32)
    b_proj = np.asarray(b_proj, dtype=np.float32)

    qmax = np.float32(127.0)
    s_x = np.maximum(np.max(np.abs(x)) / qmax, np.float32(1e-8))
    s_wq = np.maximum(np.max(np.abs(w_qkv)) / qmax, np.float32(1e-8))
    s_wp = np.maximum(np.max(np.abs(w_proj)) / qmax, np.float32(1e-8))
    inv_s_x = float(np.float32(1.0) / s_x)

    wq_qkv = np.round(w_qkv / s_wq).astype(np.int8)
    wq_proj = np.round(w_proj / s_wp).astype(ml_dtypes.bfloat16)
    bqs = (b_qkv / (s_x * s_wq)).astype(np.float32)[None, :]
    bp = b_proj.astype(np.float32)[None, :]

    sxw = float(s_x) * float(s_wq)
    sfin = float(s_wp) * sxw / (127.0**3)
    catt = 0.125 * sxw * sxw / (127.0 * 127.0)
    inv32 = np.float32(inv_s_x)
    in_maps = [
        {
            "xT": np.round(np.ascontiguousarray(x[b].T) * inv32).astype(np.int8),
            "wq_qkv": wq_qkv,
            "wq_proj": wq_proj,
            "bqs": bqs,
            "bp": bp,
        }
        for b in range(B)
    ]
    return inv_s_x, sfin, catt, in_maps


_CACHE = {}


def kernel(x, w_qkv, b_qkv, w_proj, b_proj):
    inv_s_x, sfin, catt, in_maps = _host_prep(x, w_qkv, b_qkv, w_proj, b_proj)
    key = (inv_s_x, sfin, catt)
    if key not in _CACHE:
        SFIN_CONST[0] = sfin
        CATT_CONST[0] = catt
        _CACHE[key] = build_graph(inv_s_x)
    nc = _CACHE[key]
    res = run_bass_kernel_spmd(nc, in_maps, CORES)
    return np.stack([res.results[b]["out"] for b in range(B)], axis=0)


def build_and_inmaps(x, w_qkv, b_qkv, w_proj, b_proj):
    inv_s_x, sfin, catt, in_maps = _host_prep(x, w_qkv, b_qkv, w_proj, b_proj)
    SFIN_CONST[0] = sfin
    CATT_CONST[0] = catt
    nc = build_graph(inv_s_x)
    return nc, in_maps


# revision 15
# speedup vs baseline: 1.4973x; 1.0190x over previous
"""Fake-quantized multi-head attention block on 8 TRN2 NeuronCores.

Data-parallel over batch (1 element per core); integer-domain quantized
matmuls in bf16; global fake-quant scales via tiny AllGather
collectives + local max.  Key structural points:

- A dummy AllGather fires at t=0 to absorb SPMD launch skew while
  stage-1 runs; later collectives then cost ~2-3us instead of ~12-27us.
- aug mega-tensors [128, 12*N]: rows 0:64 = q/k head slices (ints),
  aug_k rows 64:66 = ones, aug_q rows 64:66 = hi/lo split of
  ln(r*127/maxp)/catt written after AG#2, rows 66:128 zero.  Full
  128-row operands in BOTH attention phases keep the PE HAM clock-gate
  at full speed (64-row operands measurably halve the PE clock).
- v absmax rides its own early AllGather (AG#v) fired before phase A;
  v-quantize runs on gpsimd during phase A (gpsimd is otherwise idle).
- ln(r)/catt is computed and PE-transposed during AG#2 flight; the
  global -ln(maxp/127)/catt constant lands as one ACT bias add, and the
  hi/lo rows land in the aug tile with 2 strided DMAs.
"""

import sys

sys.path.insert(0, "/opt/trn_rl_repo")

import numpy as np
import ml_dtypes

import concourse.mybir as mybir
import concourse.tile as tile
import concourse.bass_isa as bass_isa
from concourse import bacc
from concourse.bass_utils import run_bass_kernel_spmd

f32 = mybir.dt.float32
bf16 = mybir.dt.bfloat16
i8 = mybir.dt.int8
ALU = mybir.AluOpType
ACT = mybir.ActivationFunctionType
AX = mybir.AxisListType
RED = bass_isa.ReduceOp

B, N, C = 8, 1024, 768
H, HD = 12, 64
NCORES = 8
MAGIC = float(np.float32(3 * 2**22))
CORES = list(range(NCORES))
RG = [CORES]

SFIN_CONST = [1.0]
CATT_CONST = [1.0]


def build_graph(inv_s_x: float):
    nc = bacc.Bacc("TRN2", target_bir_lowering=False, debug=False, num_devices=NCORES)

    xT_ext = nc.dram_tensor("xT", [C, N], i8, kind="ExternalInput")
    wq_qkv_ext = nc.dram_tensor("wq_qkv", [C, 3 * C], i8, kind="ExternalInput")
    wq_proj_ext = nc.dram_tensor("wq_proj", [C, C], bf16, kind="ExternalInput")
    bqs_ext = nc.dram_tensor("bqs", [1, 3 * C], f32, kind="ExternalInput")
    bp_ext = nc.dram_tensor("bp", [1, C], f32, kind="ExternalInput")
    out_ext = nc.dram_tensor("out", [N, C], f32, kind="ExternalOutput")

    with tile.TileContext(nc) as tc:
        run_body(nc, tc, inv_s_x, xT_ext, wq_qkv_ext, wq_proj_ext, bqs_ext, bp_ext, out_ext)
    nc.finalize()
    return nc


def run_body(nc, tc, inv_s_x, xT_ext, wq_qkv_ext, wq_proj_ext, bqs_ext, bp_ext, out_ext):
    with (
        tc.tile_pool(name="persist", bufs=1) as pp,
        tc.tile_pool(name="dram", bufs=1, space="DRAM") as dram,
    ):
        # aug mega-tensors: column block h*N..(h+1)*N is head h
        aug_q = pp.tile([128, H * N], bf16, tag="aug_q", name="aug_q")
        aug_k = pp.tile([128, H * N], bf16, tag="aug_k", name="aug_k")
        zbuf = pp.tile([128, 96], f32, tag="zbuf", name="zbuf")
        mlbuf = pp.tile([128, 96], f32, tag="mlbuf", name="mlbuf")
        qkv_abs = pp.tile([128, 24], f32, tag="qkv_abs", name="qkv_abs")
        v_abs = pp.tile([128, 16], f32, tag="v_abs", name="v_abs")
        o_abs = pp.tile([128, 6], f32, tag="o_abs", name="o_abs")
        sc = pp.tile([128, 16], f32, tag="sc", name="sc")
        bqs_cols = pp.tile([128, 12], f32, tag="bqs_cols", name="bqs_cols")
        bv_bc = pp.tile([128, C], f32, tag="bv_bc", name="bv_bc")
        hi_rows = pp.tile([H, N], bf16, tag="hi_rows", name="hi_rows")
        lo_rows = pp.tile([H, N], bf16, tag="lo_rows", name="lo_rows")
        magic_col = pp.tile([128, 1], f32, tag="magic_col", name="magic_col")
        mpacc = pp.tile([128, 1], f32, tag="mpacc", name="mpacc")
        nmagic_col = pp.tile([128, 1], f32, tag="nmagic_col", name="nmagic_col")

        ar0_in = dram.tile([1, 1], f32, tag="ar0_in", name="ar0_in")
        ar0_out = dram.tile([1, 8], f32, tag="ar0_out", name="ar0_out")
        arm2_in = dram.tile([1, 1], f32, tag="arm2_in", name="arm2_in")
        arm2_out = dram.tile([1, 8], f32, tag="arm2_out", name="arm2_out")
        ar1_in = dram.tile([1, 2], f32, tag="ar1_in", name="ar1_in")
        ar1_out = dram.tile([1, 2], f32, tag="ar1_out", name="ar1_out")
        arv_in = dram.tile([1, 1], f32, tag="arv_in", name="arv_in")
        arv_out = dram.tile([1, 1], f32, tag="arv_out", name="arv_out")
        ar2_in = dram.tile([1, 1], f32, tag="ar2_in", name="ar2_in")
        ar2_out = dram.tile([1, 1], f32, tag="ar2_out", name="ar2_out")
        ar3_in = dram.tile([1, 1], f32, tag="ar3_in", name="ar3_in")
        ar3_out = dram.tile([1, 1], f32, tag="ar3_out", name="ar3_out")

        nc.vector.memset(magic_col[:], MAGIC)
        nc.vector.memset(mpacc[:], 0.0)

        # per-output-channel qkv bias columns: one strided DMA, issued
        # before anything else queues on gpsimd
        nc.gpsimd.dma_start(
            bqs_cols[:, 0:12],
            bqs_ext[0:1, 0:1536].rearrange("a (t p) -> (a p) t", p=128),
        )

        # PE warm-up burst (ramp the PE clock) while DMAs fly
        wut = pp.tile([128, 512], bf16, tag="wut", name="wut")
        nc.vector.memset(wut[:], 1.0)
        with tc.tile_pool(name="pswu", bufs=1, space="PSUM") as pswu:
            wps = pswu.tile([128, 512], f32, tag="wps", name="wps")
            for _ in range(24):
                nc.tensor.matmul(wps[:], lhsT=wut[:, 0:128], rhs=wut[:], start=True, stop=True)
        nc.vector.memset(nmagic_col[:], -MAGIC)
        with tc.tile_pool(name="brow", bufs=1) as br:
            bvrow = br.tile([1, C], f32, tag="bvrow", name="bvrow")
            nc.sync.dma_start(bvrow[:], bqs_ext[0:1, 1536:2304])
            nc.gpsimd.partition_broadcast(bv_bc[:], bvrow[:])

        # late pool: tensors born mid-kernel (vq during phase A, bp_bc at proj)
        with tc.tile_pool(name="late", bufs=1) as lp:
         vq = [lp.tile([128, C], bf16, tag=f"vq{t}", name=f"vq{t}") for t in range(8)]
         bp_bc = lp.tile([128, C], f32, tag="bp_bc", name="bp_bc")
         # v_f persists until v-quant (on gpsimd during phase A)
         with tc.tile_pool(name="vf_pool", bufs=1) as vfp:
          v_f = [vfp.tile([128, C], f32, tag=f"vf{t}", name=f"vf{t}") for t in range(8)]

          # ---- stage 1+2: x quant, QKV matmuls, AG#1 (q,k), quantize -----
          with (
            tc.tile_pool(name="wload", bufs=1) as wl,
            tc.tile_pool(name="qkvf_pool", bufs=1) as qp,
            tc.tile_pool(name="s12", bufs=2) as s12,
            tc.tile_pool(name="psq", bufs=4, space="PSUM") as psq,
          ):
            wq_bf = [wl.tile([128, 3 * C], bf16, tag=f"wq{t}", name=f"wq{t}") for t in range(6)]
            xq = [wl.tile([128, N], bf16, tag=f"xq{t}", name=f"xq{t}") for t in range(6)]
            qkv_f = [qp.tile([128, N], f32, tag=f"qkvf{t}", name=f"qkvf{t}") for t in range(12)]

            # int8 inputs: DMA then upcast to bf16 (x-quant happened on host)
            for t in range(6):
                x8 = wl.tile([128, N], i8, tag="x8", name=f"x8{t}", bufs=2)
                wq8 = wl.tile([128, 3 * C], i8, tag="wq8", name=f"wq8{t}", bufs=2)
                nc.sync.dma_start(x8[:], xT_ext[t * 128 : (t + 1) * 128, :])
                nc.sync.dma_start(wq8[:], wq_qkv_ext[t * 128 : (t + 1) * 128, :])
                nc.vector.tensor_copy(xq[t][:], x8[:])
                nc.vector.tensor_copy(wq_bf[t][:], wq8[:])

            # q/k part: per-mt accumulation over kt
            for mt in range(12):
                pss = [psq.tile([128, 512], f32, tag="psq", name="psq", bufs=6) for _ in range(2)]
                for kt in range(6):
                    for nk in range(2):
                        mi = nc.tensor.matmul(
                            pss[nk][:],
                            lhsT=wq_bf[kt][:, mt * 128 : (mt + 1) * 128],
                            rhs=xq[kt][:, nk * 512 : (nk + 1) * 512],
                            start=(kt == 0),
                            stop=(kt == 5),
                            skip_group_check=True,
                        )
                        if nk == 1:
                            mi.ins.ldweights = False
                for nk in range(2):
                    nc.scalar.activation(
                        qkv_f[mt][:, nk * 512 : (nk + 1) * 512],
                        pss[nk][:],
                        ACT.Identity,
                        bias=bqs_cols[:, mt : mt + 1],
                    )
                    nc.vector.tensor_reduce(
                        qkv_abs[:, mt * 2 + nk : mt * 2 + nk + 1],
                        qkv_f[mt][:, nk * 512 : (nk + 1) * 512],
                        axis=AX.X,
                        op=ALU.max,
                        apply_absolute_value=True,
                    )

            # ---- AG#1: global absmax of q, k --------------------------------
            am2 = s12.tile([128, 2], f32, tag="am2", name="am2")
            nc.vector.tensor_reduce(am2[:, 0:1], qkv_abs[:, 0:12], axis=AX.X, op=ALU.max)
            nc.vector.tensor_reduce(am2[:, 1:2], qkv_abs[:, 12:24], axis=AX.X, op=ALU.max)
            am2r = s12.tile([128, 2], f32, tag="am2r", name="am2r")
            nc.gpsimd.partition_all_reduce(am2r[:], am2[:], 128, RED.max)
            nc.gpsimd.dma_start(ar1_in[:], am2r[0:1, :])
            nc.gpsimd.collective_compute(
                "AllReduce", ALU.max, replica_groups=RG, ins=[ar1_in.opt()], outs=[ar1_out.opt()]
            )
            g2 = pp.tile([1, 2], f32, tag="g2", name="g2")
            nc.gpsimd.dma_start(g2[:], ar1_out[0:1, 0:2])
            g2b = pp.tile([128, 2], f32, tag="g2b", name="g2b")
            nc.gpsimd.partition_broadcast(g2b[:], g2[:])

            # aug zero/one fills: per-head chunks, emitted after the AG#1
            # fire so they don't clog the gpsimd queue ahead of it
            for h in range(H):
                nc.gpsimd.memset(aug_q[64:128, h * N : (h + 1) * N], 0.0)
                nc.gpsimd.memset(aug_k[64:128, h * N : (h + 1) * N], 0.0)
                nc.gpsimd.memset(aug_k[64:66, h * N : (h + 1) * N], 1.0)

            inv2 = pp.tile([128, 2], f32, tag="inv2", name="inv2")
            nc.vector.reciprocal(inv2[:], g2b[:])
            nc.vector.tensor_scalar(inv2[:], inv2[:], 127.0, None, ALU.mult)
            nc.vector.tensor_tensor(sc[:, 3:4], g2b[:, 0:1], g2b[:, 1:2], ALU.mult)
            nc.vector.tensor_scalar(sc[:, 3:4], sc[:, 3:4], CATT_CONST[0], None, ALU.mult)
            nc.vector.reciprocal(sc[:, 9:10], sc[:, 3:4])

            # v matmuls fill the PE idle window during/after AG#1 (their
            # absmax rides the separate AG#v)
            for nt in range(8):
                pss = []
                for ick, (ck, cw) in enumerate(((0, 512), (512, 256))):
                    pss.append((psq.tile([128, 512], f32, tag="psq", name="psv", bufs=6), ck, cw))
                for kt in range(6):
                    for ick2, (ps, ck, cw) in enumerate(pss):
                        mi = nc.tensor.matmul(
                            ps[:, 0:cw],
                            lhsT=xq[kt][:, nt * 128 : (nt + 1) * 128],
                            rhs=wq_bf[kt][:, 1536 + ck : 1536 + ck + cw],
                            start=(kt == 0),
                            stop=(kt == 5),
                            skip_group_check=True,
                        )
                        if ick2 == 1:
                            mi.ins.ldweights = False
                for ick, (ps, ck, cw) in enumerate(pss):
                    nc.vector.scalar_tensor_tensor(
                        v_f[nt][:, ck : ck + cw],
                        ps[:, 0:cw],
                        1.0,
                        bv_bc[:, ck : ck + cw],
                        ALU.mult,
                        ALU.add,
                    )
                    nc.vector.tensor_reduce(
                        v_abs[:, nt * 2 + ick : nt * 2 + ick + 1],
                        v_f[nt][:, ck : ck + cw],
                        axis=AX.X,
                        op=ALU.max,
                        apply_absolute_value=True,
                    )

            # ---- AG#v: global absmax of v (fire before phase A) -------------
            vam = s12.tile([128, 1], f32, tag="vam", name="vam")
            nc.vector.tensor_reduce(vam[:], v_abs[:], axis=AX.X, op=ALU.max)
            vamr = s12.tile([128, 1], f32, tag="vamr", name="vamr")
            nc.gpsimd.partition_all_reduce(vamr[:], vam[:], 128, RED.max)
            nc.gpsimd.dma_start(arv_in[:], vamr[0:1, :])
            nc.gpsimd.collective_compute(
                "AllReduce", ALU.max, replica_groups=RG, ins=[arv_in.opt()], outs=[arv_out.opt()]
            )
            gv = pp.tile([1, 1], f32, tag="gv", name="gv")
            nc.gpsimd.dma_start(gv[:], arv_out[0:1, 0:1])
            nc.gpsimd.partition_broadcast(sc[:, 14:15], gv[:])
            nc.vector.reciprocal(sc[:, 15:16], sc[:, 14:15])
            nc.vector.tensor_scalar(sc[:, 15:16], sc[:, 15:16], 127.0, None, ALU.mult)

            # PE keep-warm through the AR#1 result window
            for wi in range(16):
                wq_warm = psq.tile([128, 512], f32, tag="psq", name=f"warm1_{wi}", bufs=6)
                nc.tensor.matmul(wq_warm[:], lhsT=wut[:, 0:128], rhs=wut[:], start=True, stop=True)

            # ---- quantize q/k into the aug mega-tiles ----------------------
            for i, mt in enumerate((0, 6, 1, 7, 2, 8, 3, 9, 4, 10, 5, 11)):
                inv = inv2[:, 0:1] if mt < 6 else inv2[:, 1:2]
                y = s12.tile([128, N], f32, tag="s12y", name="yq", bufs=2)
                nc.scalar.activation(y[:], qkv_f[mt][:], ACT.Identity, bias=magic_col[:], scale=inv)
                qsc = s12.tile([128, N], bf16, tag="qsc", name="qsc", bufs=2)
                nc.vector.tensor_scalar(qsc[:], y[:], MAGIC, None, ALU.subtract)
                dst = aug_q if mt < 6 else aug_k
                tt = mt if mt < 6 else mt - 6
                nc.sync.dma_start(dst[0:64, (2 * tt) * N : (2 * tt + 1) * N], qsc[0:64, :])
                nc.sync.dma_start(dst[0:64, (2 * tt + 1) * N : (2 * tt + 2) * N], qsc[64:128, :])


          # ---- phase A: attn[i,j] logits stats -----------------------------
          with (
            tc.tile_pool(name="phA", bufs=4) as pa,
            tc.tile_pool(name="psA", bufs=3, space="PSUM") as psa,
          ):
            armid_dummy = pp.tile([1, 8], f32, tag="armid_dummy", name="armid_dummy")
            for h in range(H):
                if h == 6:
                    # resync collective: input depends on head-5 stats so the
                    # fire can't be hoisted before mid-phase-A; absorbs
                    # inter-core drift under the compute shadow
                    nc.gpsimd.dma_start(ar0_in[:], zbuf[0:1, 47:48])
                    nc.gpsimd.collective_compute(
                        "AllGather", ALU.bypass, replica_groups=RG,
                        ins=[ar0_in.opt()], outs=[ar0_out.opt()],
                    )
                if h == 9:
                    nc.gpsimd.dma_start(armid_dummy[:], ar0_out[0:1, :])
                for it in range(8):
                    psl = psa.tile([128, N], f32, tag="psl", name="psl")
                    for jc in range(2):
                        mi = nc.tensor.matmul(
                            psl[:, jc * 512 : (jc + 1) * 512],
                            lhsT=aug_q[:, h * N + it * 128 : h * N + (it + 1) * 128],
                            rhs=aug_k[:, h * N + jc * 512 : h * N + (jc + 1) * 512],
                            start=True,
                            stop=True,
                        )
                        if jc == 1:
                            mi.ins.ldweights = False
                    col = h * 8 + it
                    ea = pa.tile([128, N], bf16, tag="ea", name="ea")
                    nc.scalar.activation(
                        ea[:], psl[:], ACT.Exp, scale=sc[:, 3:4],
                        accum_out=zbuf[:, col : col + 1],
                    )
                    # maxe directly from ea (exp is monotone): bf16 read is 2x
                    nc.vector.tensor_reduce(mlbuf[:, col : col + 1], ea[:], axis=AX.X, op=ALU.max)
                    wps2 = psa.tile([128, 512], f32, tag="wps2", name="wps2", bufs=2)
                    nc.tensor.matmul(wps2[:], lhsT=wut[:, 0:128], rhs=ea[:, 0:512], start=True, stop=True)
                # incremental max-prob: mpacc = max(mpacc, rowmax(maxe_h / z_h))
                rzh = pa.tile([128, 8], f32, tag="rzh", name=f"rzh{h}", bufs=2)
                nc.vector.reciprocal(rzh[:], zbuf[:, h * 8 : (h + 1) * 8])
                mph = pa.tile([128, 8], f32, tag="mph", name=f"mph{h}", bufs=2)
                nc.vector.tensor_tensor(mph[:], mlbuf[:, h * 8 : (h + 1) * 8], rzh[:], ALU.mult)
                mphr = pa.tile([128, 1], f32, tag="mphr", name=f"mphr{h}", bufs=2)
                nc.vector.tensor_reduce(mphr[:], mph[:], axis=AX.X, op=ALU.max)
                nc.vector.tensor_tensor(mpacc[:], mpacc[:], mphr[:], ALU.max)

          # ---- AG#2: max prob; ln(r)/c rows --------------------------------
          with (
            tc.tile_pool(name="phR", bufs=1) as pr,
            tc.tile_pool(name="psT", bufs=1, space="PSUM") as pst,
          ):
            from concourse.masks import make_identity

            pk1r = pr.tile([128, 1], f32, tag="pk1r", name="pk1r")
            nc.gpsimd.partition_all_reduce(pk1r[:], mpacc[:], 128, RED.max)
            nc.gpsimd.dma_start(ar2_in[:], pk1r[0:1, :])
            nc.gpsimd.collective_compute(
                "AllReduce", ALU.max, replica_groups=RG, ins=[ar2_in.opt()], outs=[ar2_out.opt()]
            )
            rz = pr.tile([128, 96], f32, tag="rz", name="rz")
            nc.vector.reciprocal(rz[:], zbuf[:])

            # -- PE keep-warm during AG#2 window ----------------------------
            wpsw = pst.tile([128, 512], f32, tag="wpsw", name="wpsw", bufs=2)
            wpsw2 = pst.tile([128, 512], f32, tag="wpsw", name="wpsw2", bufs=2)
            for wi in range(8):
                nc.tensor.matmul(
                    (wpsw if wi % 2 == 0 else wpsw2)[:],
                    lhsT=wut[:, 0:128], rhs=wut[:], start=True, stop=True,
                )

            # -- during AG#2 flight: v-quant (needs only AG#v result) --------
            for nt in range(8):
                yv = pr.tile([128, C], f32, tag="yv", name="yv", bufs=2)
                nc.scalar.activation(yv[:], v_f[nt][:], ACT.Identity, bias=magic_col[:], scale=sc[:, 15:16])
                nc.vector.tensor_scalar(vq[nt][:], yv[:], MAGIC, None, ALU.subtract)

            # -- during AG#2 flight: ln(r)/catt, transposed ------------------
            lnr = pr.tile([128, 96], f32, tag="lnr", name="lnr")
            nc.scalar.activation(lnr[:], rz[:], ACT.Ln)
            lnrc = pr.tile([128, 128], f32, tag="lnrc", name="lnrc")
            nc.vector.memset(lnrc[:], 0.0)
            nc.vector.tensor_scalar(lnrc[:, 0:96], lnr[:], sc[:, 9:10], None, ALU.mult)
            idn = pr.tile([128, 128], f32, tag="idn", name="idn")
            make_identity(nc, idn[:])
            psT = pst.tile([128, 128], f32, tag="psT", name="psT")
            nc.tensor.transpose(psT[:], lnrc[:], idn[:])
            lnrcT_pre = pr.tile([128, 128], f32, tag="lnrcT_pre", name="lnrcT_pre")
            nc.scalar.activation(lnrcT_pre[:], psT[:], ACT.Copy)

            # -- AG#2 result: maxp_g ----------------------------------------
            g2p = pr.tile([1, 1], f32, tag="g2p", name="g2p")
            nc.gpsimd.dma_start(g2p[:], ar2_out[0:1, 0:1])
            nc.gpsimd.partition_broadcast(sc[:, 7:8], g2p[:])
            nc.vector.reciprocal(sc[:, 8:9], sc[:, 7:8])
            nc.vector.tensor_scalar(sc[:, 8:9], sc[:, 8:9], 127.0, None, ALU.mult)
            # one strided DMA: [96,128] -> [12, 1024]; runs during AG#2 flight
            lnrc_rows = pr.tile([H, N], f32, tag="lnrc_rows", name="lnrc_rows")
            nc.scalar.dma_start(lnrc_rows[:], lnrcT_pre[0:96, 0:128])
            # cterm = ln(127/maxp)/catt folded into the hi/lo split directly
            cterm = pr.tile([128, 1], f32, tag="cterm", name="cterm")
            nc.scalar.activation(cterm[:], sc[:, 8:9], ACT.Ln)
            nc.vector.tensor_tensor(cterm[:], cterm[:], sc[:, 9:10], ALU.mult)
            nc.vector.tensor_scalar(hi_rows[:], lnrc_rows[:], cterm[0:H, 0:1], None, ALU.add)
            nc.vector.scalar_tensor_tensor(
                lo_rows[:], lnrc_rows[:], cterm[0:H, 0:1], hi_rows[:], ALU.add, ALU.subtract
            )
            # two strided DMAs land hi/lo into aug_q rows 64/65
            nc.scalar.dma_start(aug_q[64:65, :], hi_rows[:])
            nc.scalar.dma_start(aug_q[65:66, :], lo_rows[:])

         # ---- phase B: quantized probs + PV (zero-padded vz operands) ----
         with tc.tile_pool(name="oint_pool", bufs=1) as op_:
           o_int = [op_.tile([128, N], f32, tag=f"oint{t}", name=f"oint{t}") for t in range(6)]
           wp_bf = [op_.tile([128, C], bf16, tag=f"wp{t}", name=f"wp{t}") for t in range(6)]
           # prefetch proj weights + bias during phase B (gpsimd queue)
           for t in range(6):
               nc.gpsimd.dma_start(wp_bf[t][:], wq_proj_ext[t * 128 : (t + 1) * 128, :])
           with tc.tile_pool(name="brow2", bufs=1) as br2:
               bprow = br2.tile([1, C], f32, tag="bprow", name="bprow")
               nc.gpsimd.dma_start(bprow[:], bp_ext[:])
               nc.gpsimd.partition_broadcast(bp_bc[:], bprow[:])
           with (
             tc.tile_pool(name="phB", bufs=4) as pb,
             tc.tile_pool(name="vzp", bufs=2) as vzp,
             tc.tile_pool(name="psB", bufs=2, space="PSUM") as psb,
             tc.tile_pool(name="psO", bufs=2, space="PSUM") as pso_pool,
           ):
             armid2_dummy = pp.tile([1, 8], f32, tag="armid2_dummy", name="armid2_dummy")
             for hp in range(6):
                 if hp == 3:
                     nc.gpsimd.dma_start(arm2_in[:], o_abs[0:1, 2:3])
                     nc.gpsimd.collective_compute(
                         "AllGather", ALU.bypass, replica_groups=RG,
                         ins=[arm2_in.opt()], outs=[arm2_out.opt()],
                     )
                 if hp == 5:
                     nc.gpsimd.dma_start(armid2_dummy[:], arm2_out[0:1, :])
                 h0, h1 = 2 * hp, 2 * hp + 1
                 # padded PV weights: vz[:, jt*256 + 0:64] = v cols of h0,
                 # vz[:, jt*256 + 192:256] = v cols of h1, rest zero.
                 vz = vzp.tile([128, 8 * 256], bf16, tag="vz", name="vz")
                 nc.gpsimd.memset(vz[:], 0.0)
                 for jt in range(8):
                     nc.sync.dma_start(
                         vz[:, jt * 256 : jt * 256 + 64], vq[jt][:, h0 * 64 : (h0 + 1) * 64]
                     )
                     nc.sync.dma_start(
                         vz[:, jt * 256 + 192 : jt * 256 + 256], vq[jt][:, h1 * 64 : (h1 + 1) * 64]
                     )
                 pso = pso_pool.tile([128, N], f32, tag="pso", name="pso")
                 for jt in range(8):
                     pqs = []
                     for h in (h0, h1):
                         pslT = psb.tile([128, N], f32, tag="pslT", name="pslT")
                         for ic in range(2):
                             mi = nc.tensor.matmul(
                                 pslT[:, ic * 512 : (ic + 1) * 512],
                                 lhsT=aug_k[:, h * N + jt * 128 : h * N + (jt + 1) * 128],
                                 rhs=aug_q[:, h * N + ic * 512 : h * N + (ic + 1) * 512],
                                 start=True,
                                 stop=True,
                             )
                             if ic == 1:
                                 mi.ins.ldweights = False
                         ep = pb.tile([128, N], f32, tag="ep", name="ep")
                         nc.scalar.activation(ep[:], pslT[:], ACT.Exp, scale=sc[:, 3:4])
                         pq = pb.tile([128, N], bf16, tag="pq", name="pq")
                         nc.vector.tensor_scalar(pq[:], ep[:], MAGIC, MAGIC, ALU.add, ALU.subtract)
                         pqs.append(pq)
                     for hh, pq in enumerate(pqs):
                         for ic in range(2):
                             mi = nc.tensor.matmul(
                                 pso[:, ic * 512 : (ic + 1) * 512],
                                 lhsT=vz[:, jt * 256 + hh * 128 : jt * 256 + (hh + 1) * 128],
                                 rhs=pq[:, ic * 512 : (ic + 1) * 512],
                                 start=(jt == 0 and hh == 0),
                                 stop=(jt == 7 and hh == 1),
                                 skip_group_check=True,
                             )
                             if ic == 1:
                                 mi.ins.ldweights = False
                 nc.vector.tensor_copy(o_int[hp][:], pso[:])
                 nc.vector.tensor_reduce(
                     o_abs[:, hp : hp + 1], o_int[hp][:], axis=AX.X, op=ALU.max, apply_absolute_value=True
                 )

           # ---- AG#3 + quantize o + proj ----------------------------------
           with (
             tc.tile_pool(name="phC", bufs=3) as pc,
             tc.tile_pool(name="oq_pool", bufs=1) as oqp,
             tc.tile_pool(name="psF", bufs=4, space="PSUM") as psf_pool,
           ):
             oam = pc.tile([128, 1], f32, tag="oam", name="oam")
             nc.vector.tensor_reduce(oam[:], o_abs[:], axis=AX.X, op=ALU.max)
             oamr = pc.tile([128, 1], f32, tag="oamr", name="oamr")
             nc.gpsimd.partition_all_reduce(oamr[:], oam[:], 128, RED.max)
             nc.gpsimd.dma_start(ar3_in[:], oamr[0:1, :])
             nc.gpsimd.collective_compute(
                 "AllReduce", ALU.max, replica_groups=RG, ins=[ar3_in.opt()], outs=[ar3_out.opt()]
             )
             g3 = pc.tile([1, 1], f32, tag="g3", name="g3")
             nc.gpsimd.dma_start(g3[:], ar3_out[0:1, 0:1])
             nc.gpsimd.partition_broadcast(sc[:, 10:11], g3[:])

             nc.vector.reciprocal(sc[:, 11:12], sc[:, 10:11])
             nc.vector.tensor_scalar(sc[:, 11:12], sc[:, 11:12], 127.0, None, ALU.mult)
             nc.vector.tensor_tensor(sc[:, 12:13], sc[:, 7:8], sc[:, 14:15], ALU.mult)
             nc.vector.tensor_tensor(sc[:, 12:13], sc[:, 12:13], sc[:, 10:11], ALU.mult)
             nc.vector.tensor_scalar(sc[:, 12:13], sc[:, 12:13], SFIN_CONST[0], None, ALU.mult)

             oq = [oqp.tile([128, N], bf16, tag=f"oq{t}", name=f"oq{t}") for t in range(6)]
             for t in range(6):
                 if t % 2 == 0:
                     y = pc.tile([128, N], f32, tag="yo", name="yo")
                     nc.scalar.activation(y[:], o_int[t][:], ACT.Identity, bias=magic_col[:], scale=sc[:, 11:12])
                     nc.vector.tensor_scalar(oq[t][:], y[:], MAGIC, None, ALU.subtract)
                 else:
                     y = pc.tile([128, N], f32, tag="yo", name="yo")
                     nc.vector.tensor_scalar(y[:], o_int[t][:], sc[:, 11:12], MAGIC, ALU.mult, ALU.add)
                     nc.vector.tensor_scalar(oq[t][:], y[:], MAGIC, None, ALU.subtract)

             for g in range(2):
                 psfs = [psf_pool.tile([128, C], f32, tag="psf", name="psf") for _ in range(4)]
                 if g == 0:
                     for wi in range(20):
                         nc.tensor.matmul(
                             psfs[wi % 2][:, 0:512],
                             lhsT=wut[:, 0:128], rhs=wut[:], start=True, stop=True,
                         )
                 for kt in range(6):
                     for nn in range(4):
                         nt = g * 4 + nn
                         for ick2, (ck, cw) in enumerate(((0, 512), (512, 256))):
                             mi = nc.tensor.matmul(
                                 psfs[nn][:, ck : ck + cw],
                                 lhsT=oq[kt][:, nt * 128 : (nt + 1) * 128],
                                 rhs=wp_bf[kt][:, ck : ck + cw],
                                 start=(kt == 0),
                                 stop=(kt == 5),
                                 skip_group_check=True,
                             )
                             if ick2 == 1:
                                 mi.ins.ldweights = False
                 for nn in range(4):
                     nt = g * 4 + nn
                     ot = pc.tile([128, C], f32, tag="ot", name="ot")
                     nc.vector.scalar_tensor_tensor(
                         ot[:], psfs[nn][:], sc[:, 12:13], bp_bc[:], ALU.mult, ALU.add
                     )
                     nc.sync.dma_start(out_ext[nt * 128 : (nt + 1) * 128, :], ot[:])


def _host_prep(x, w_qkv, b_qkv, w_proj, b_proj):
    x = np.asarray(x, dtype=np.float32)
    w_qkv = np.asarray(w_qkv, dtype=np.float32)
    b_qkv = np.asarray(b_qkv, dtype=np.float32)
    w_proj = np.asarray(w_proj, dtype=np.float32)
    b_proj = np.asarray(b_proj, dtype=np.float32)

    qmax = np.float32(127.0)
    s_x = np.maximum(np.max(np.abs(x)) / qmax, np.float32(1e-8))
    s_wq = np.maximum(np.max(np.abs(w_qkv)) / qmax, np.float32(1e-8))
    s_wp = np.maximum(np.max(np.abs(w_proj)) / qmax, np.float32(1e-8))
    inv_s_x = float(np.float32(1.0) / s_x)

    wq_qkv = np.round(w_qkv / s_wq).astype(np.int8)
    wq_proj = np.round(w_proj / s_wp).astype(ml_dtypes.bfloat16)
    bqs = (b_qkv / (s_x * s_wq)).astype(np.float32)[None, :]
    bp = b_proj.astype(np.float32)[None, :]

    sxw = float(s_x) * float(s_wq)
    sfin = float(s_wp) * sxw / (127.0**3)
    catt = 0.125 * sxw * sxw / (127.0 * 127.0)
    inv32 = np.float32(inv_s_x)
    in_maps = [
        {
            "xT": np.round(np.ascontiguousarray(x[b].T) * inv32).astype(np.int8),
            "wq_qkv": wq_qkv,
            "wq_proj": wq_proj,
            "bqs": bqs,
            "bp": bp,
        }
        for b in range(B)
    ]
    return inv_s_x, sfin, catt, in_maps


_CACHE = {}


def kernel(x, w_qkv, b_qkv, w_proj, b_proj):
    inv_s_x, sfin, catt, in_maps = _host_prep(x, w_qkv, b_qkv, w_proj, b_proj)
    key = (inv_s_x, sfin, catt)
    if key not in _CACHE:
        SFIN_CONST[0] = sfin
        CATT_CONST[0] = catt
        _CACHE[key] = build_graph(inv_s_x)
    nc = _CACHE[key]
    res = run_bass_kernel_spmd(nc, in_maps, CORES)
    return np.stack([res.results[b]["out"] for b in range(B)], axis=0)


def build_and_inmaps(x, w_qkv, b_qkv, w_proj, b_proj):
    inv_s_x, sfin, catt, in_maps = _host_prep(x, w_qkv, b_qkv, w_proj, b_proj)
    SFIN_CONST[0] = sfin
    CATT_CONST[0] = catt
    nc = build_graph(inv_s_x)
    return nc, in_maps
